# revision 2
# baseline (speedup 1.0000x reference)
"""MLA forward on 8 trn2 cores — absorbed-weight tensor-parallel version.

Key algebraic move: RMSNorm(z) = z * inv_rms(z) * w with inv_rms a per-token
scalar, so the LoRA up-projections absorb the down-projections on the host:
  q   = (x @ (Wqb diag(w) Wqa)^T) * inv_rms(x Wqa^T)
  k,v = (x @ (Wkvb diag(w) Wkva_c)^T) * inv_rms(x Wkva_c^T)
Each core only computes the absorbed GEMMs for its 2 heads instead of the
replicated LoRA-A GEMMs. The per-token inv_rms scalars still need the full
latent rows; those are computed seq-sharded (256 tokens per core) and
exchanged with a single 16 KB AllGather whose latency hides under the
absorbed GEMMs. inv_rms scaling is deferred until after the gather (rope
commutes with per-token scaling).

Precision plan (tolerance 2e-2, validated offline in fp8_sim.py):
- stats GEMMs: fp8e4 DoubleRow, 1-term (inv_rms only needs ~0.2%)
- absorbed GEMMs: fp8e4 DoubleRow, 3-term compensated
  (x8@w_hi + r8@w_hi + x8@w_lo), host-prepared splits
- attention + o_proj: bf16 operands, fp32 PSUM
Weights are pre-scaled by 2^5 to center e4m3; the factor is folded into the
Sqrt activation scale (stats), the Exp scale (q·k carries 2^10), and into
w_o on the host (v path). Softmax denominators accumulate on DVE. o_proj
partials are summed on the host across cores (bf16 partial writes).
"""
import numpy as np

import concourse.bass as bass
import concourse.tile as tile
from concourse import bacc, mybir
from concourse.bass_utils import run_bass_kernel_spmd

F32 = mybir.dt.float32
F32R = mybir.dt.float32r
BF16 = mybir.dt.bfloat16
F8 = mybir.dt.float8e4

HIDDEN = 2048
S = 2048
NUM_HEADS = 16
Q_LORA = 1536
KV_LORA = 512
NOPE = 128
ROPE = 64
VD = 128
QD = NOPE + ROPE            # 192
SCALE = QD ** -0.5
EPS = 1e-6
ROPE_THETA = 10000.0

NCORES = 8
HPC = NUM_HEADS // NCORES   # 2
SB = 512
NSB = S // SB               # 4
KT = HIDDEN // 128          # 16
NJ = KT // 2                # 8 DoubleRow k-pairs
SEQB = S // NCORES          # 256 stats tokens per core
QMT = (HPC * QD) // 128     # 3
KMT = (HPC * NOPE) // 128   # 2
WS = 32.0                   # 2^5 weight pre-scale for e4m3

_CACHE = {}
LAST_RESULT = None


def _build_program():
    nc = bacc.Bacc("TRN2", target_bir_lowering=False, debug=False,
                   num_devices=NCORES)
    dt = nc.dram_tensor
    d = {
        "xs": dt("xs", [128, KT, SEQB], F8, kind="ExternalInput").ap(),
        "wqa": dt("wqa_t", [128, KT, Q_LORA], F8, kind="ExternalInput").ap(),
        "wkvac": dt("wkvac_t", [128, KT, KV_LORA], F8, kind="ExternalInput").ap(),
        "xh": dt("xh", [128, KT, S], F8, kind="ExternalInput").ap(),
        "xl": dt("xl", [128, KT, S], F8, kind="ExternalInput").ap(),
        "bth": dt("bth", [128, KT, HPC * QD], F8, kind="ExternalInput").ap(),
        "btl": dt("btl", [128, KT, HPC * QD], F8, kind="ExternalInput").ap(),
        "ckth": dt("ckth", [128, KT, HPC * NOPE], F8, kind="ExternalInput").ap(),
        "cktl": dt("cktl", [128, KT, HPC * NOPE], F8, kind="ExternalInput").ap(),
        "cvth": dt("cvth", [128, KT, HPC * VD], F8, kind="ExternalInput").ap(),
        "cvtl": dt("cvtl", [128, KT, HPC * VD], F8, kind="ExternalInput").ap(),
        "wpeh": dt("wpeh", [128, KT, ROPE], F8, kind="ExternalInput").ap(),
        "wpel": dt("wpel", [128, KT, ROPE], F8, kind="ExternalInput").ap(),
        "wo": dt("wo_t", [128, HPC, HIDDEN], BF16, kind="ExternalInput").ap(),
        "cos": dt("cosd", [128, S], BF16, kind="ExternalInput").ap(),
        "sin": dt("sind", [128, S], BF16, kind="ExternalInput").ap(),
        "msk": dt("mask", [128, 4, SB], BF16, kind="ExternalInput").ap(),
        "onesb": dt("onesb", [128, 128], BF16, kind="ExternalInput").ap(),
        "onesr": dt("onesr", [128, 128], F32R, kind="ExternalInput").ap(),
        "rotq": dt("rotq", [128, 128], BF16, kind="ExternalInput").ap(),
        "dupx": dt("dupx", [64, 128], BF16, kind="ExternalInput").ap(),
        "duprot": dt("duprot", [64, 128], BF16, kind="ExternalInput").ap(),
        "out": dt("out", [S, HIDDEN], BF16, kind="ExternalOutput").ap(),
    }
    with tile.TileContext(nc) as tc:
        _mla(tc, d)
    nc.compile()
    return nc


def _rearr(ap):
    return ap.rearrange("(t p) f -> p t f", p=128)


def _mla(tc, d):
    nc = tc.nc
    Exp = mybir.ActivationFunctionType.Exp
    Sqrt = mybir.ActivationFunctionType.Sqrt
    DR = mybir.MatmulPerfMode.DoubleRow
    QLT = Q_LORA // 128     # 12
    CT = KV_LORA // 128     # 4

    with nc.allow_low_precision(reason="fp8/bf16 matmul pipeline with fp32 "
                                "accumulation; tolerance is 2e-2"), \
         tc.tile_pool(name="pxb", bufs=2) as pxb, \
         tc.tile_pool(name="pconst", bufs=1) as pc, \
         tc.tile_pool(name="pqkv", bufs=1) as pqkv, \
         tc.tile_pool(name="pdram", bufs=1, space="DRAM") as pdram:
        # ---- DMAs in PE-consumption order: block-0 inputs lead ----
        xh = {}
        xl = {}
        xh[0] = pxb.tile([128, KT, SB], F8, tag="xh", name="xh0")
        bth = pc.tile([128, KT, HPC * QD], F8)
        xl[0] = pxb.tile([128, KT, SB], F8, tag="xl", name="xl0")
        btl = pc.tile([128, KT, HPC * QD], F8)
        for kk in (slice(0, 8), slice(8, KT)):
            nc.sync.dma_start(out=xh[0][:, kk, :], in_=d["xh"][:, kk, 0:SB])
            nc.sync.dma_start(out=bth[:, kk, :], in_=d["bth"][:, kk, :])
            nc.sync.dma_start(out=xl[0][:, kk, :], in_=d["xl"][:, kk, 0:SB])
            nc.sync.dma_start(out=btl[:, kk, :], in_=d["btl"][:, kk, :])
        ones_b = pc.tile([128, 128], BF16)
        nc.sync.dma_start(out=ones_b, in_=d["onesb"])
        ckth = pc.tile([128, KT, HPC * NOPE], F8)
        nc.sync.dma_start(out=ckth, in_=d["ckth"])
        cktl = pc.tile([128, KT, HPC * NOPE], F8)
        nc.sync.dma_start(out=cktl, in_=d["cktl"])
        wpeh = pc.tile([128, KT, ROPE], F8)
        nc.sync.dma_start(out=wpeh, in_=d["wpeh"])
        wpel = pc.tile([128, KT, ROPE], F8)
        nc.sync.dma_start(out=wpel, in_=d["wpel"])
        cvth = pc.tile([128, KT, HPC * VD], F8)
        nc.sync.dma_start(out=cvth, in_=d["cvth"])
        cvtl = pc.tile([128, KT, HPC * VD], F8)
        nc.sync.dma_start(out=cvtl, in_=d["cvtl"])
        rotq = pc.tile([128, 128], BF16)
        nc.sync.dma_start(out=rotq, in_=d["rotq"])
        dupx = pc.tile([64, 128], BF16)
        nc.sync.dma_start(out=dupx, in_=d["dupx"])
        duprot = pc.tile([64, 128], BF16)
        nc.sync.dma_start(out=duprot, in_=d["duprot"])
        cosd = pc.tile([128, S], BF16)
        nc.sync.dma_start(out=cosd, in_=d["cos"])
        sind = pc.tile([128, S], BF16)
        nc.sync.dma_start(out=sind, in_=d["sin"])
        eps1 = pc.tile([1, 1], F32)
        nc.vector.memset(eps1, EPS)

        # ---- persistent per-head tensors ----
        qn = [pqkv.tile([128, S], BF16, tag=f"qn{h}", name=f"qn{h}")
              for h in range(HPC)]
        kn = [pqkv.tile([128, S], BF16, tag=f"kn{h}", name=f"kn{h}")
              for h in range(HPC)]
        qpe = pqkv.tile([128, S], BF16, tag="qpe")
        kpd = pqkv.tile([128, S], BF16, tag="kpd")
        vst = pqkv.tile([128, S // 128, HPC * VD], BF16, tag="vst")
        ao = [pqkv.tile([128, S], BF16, tag=f"ao{h}", name=f"ao{h}")
              for h in range(HPC)]
        invqb = pqkv.tile([128, S], BF16, tag="invqb")
        invcb = pqkv.tile([128, S], BF16, tag="invcb")
        invq_row = pqkv.tile([1, NCORES, SEQB], F32R, tag="invq_row")
        invc_row = pqkv.tile([1, NCORES, SEQB], F32R, tag="invc_row")
        invc_col = pqkv.tile([128, NCORES, SEQB // 128], F32, tag="invc_col")

        cc_in = pdram.tile([2, SEQB], F32R)
        cc_out = pdram.tile([NCORES, 2, SEQB], F32R)

        with tc.tile_pool(name="p2sb", bufs=2) as p2sb, \
             tc.tile_pool(name="pp2", bufs=3, space="PSUM") as pp2, \
             tc.tile_pool(name="pp2v", bufs=2, space="PSUM") as pp2v:

            def absorbed(p, wh, wl, ws, xhb, xlb, nt=3):
                """nt-term compensated fp8 DoubleRow accumulation into psum.
                3-term: x8@w_hi + r8@w_hi + x8@w_lo; 2-term drops w_lo."""
                terms = [(xhb, wh), (xlb, wh), (xhb, wl)][:nt]
                for t, (xx, ww) in enumerate(terms):
                    for j in range(NJ):
                        nc.tensor.matmul(
                            p, ww[:, 2 * j:2 * j + 2, ws],
                            xx[:, 2 * j:2 * j + 2, :],
                            start=(t == 0 and j == 0),
                            stop=(t == nt - 1 and j == NJ - 1), perf_mode=DR)

            def block_proj(b):
                cols = bass.ts(b, SB)
                if b >= 3:
                    xh[b] = pxb.tile([128, KT, SB], F8, tag="xh", name=f"xh{b}")
                    nc.sync.dma_start(out=xh[b], in_=d["xh"][:, :, cols])
                    xl[b] = pxb.tile([128, KT, SB], F8, tag="xl", name=f"xl{b}")
                    nc.sync.dma_start(out=xl[b], in_=d["xl"][:, :, cols])

                qdst = [qn[0], qn[1]]
                for mt in range(QMT):
                    p_q = pp2.tile([128, SB], F32, tag="mm")
                    absorbed(p_q[:], bth, btl,
                             slice(mt * 128, (mt + 1) * 128), xh[b], xl[b])
                    if mt < 2:
                        nc.any.tensor_copy(qdst[mt][:, cols], p_q[:])
                    else:
                        qpe_u = p2sb.tile([128, SB], BF16, tag="t0")
                        nc.any.tensor_copy(qpe_u[:], p_q[:])
                        p_rq = pp2.tile([128, SB], F32, tag="mm")
                        nc.tensor.matmul(p_rq[:], rotq[:], qpe_u[:],
                                         start=True, stop=True)
                        t1 = p2sb.tile([128, SB], BF16, tag="t1")
                        nc.vector.tensor_mul(t1[:], qpe_u[:], cosd[:, cols])
                        t2 = p2sb.tile([128, SB], BF16, tag="t2")
                        nc.vector.tensor_mul(t2[:], p_rq[:], sind[:, cols])
                        nc.vector.tensor_add(qpe[:, cols], t1[:], t2[:])

                for mt in range(KMT):
                    p_k = pp2.tile([128, SB], F32, tag="mm")
                    absorbed(p_k[:], ckth, cktl,
                             slice(mt * 128, (mt + 1) * 128), xh[b], xl[b],
                             nt=2)
                    nc.any.tensor_copy(kn[mt][:, cols], p_k[:])

                p_pe = pp2.tile([128, SB], F32, tag="mm")
                absorbed(p_pe[:ROPE, :], wpeh, wpel, slice(0, ROPE),
                         xh[b], xl[b], nt=2)
                kpe_u = p2sb.tile([ROPE, SB], BF16, tag="t3")
                nc.any.tensor_copy(kpe_u[:], p_pe[:ROPE, :])
                p_x = pp2.tile([128, SB], F32, tag="mm")
                nc.tensor.matmul(p_x[:], dupx[:], kpe_u[:], start=True, stop=True)
                p_r = pp2.tile([128, SB], F32, tag="mm")
                nc.tensor.matmul(p_r[:], duprot[:], kpe_u[:], start=True, stop=True)
                t1 = p2sb.tile([128, SB], BF16, tag="t1")
                nc.vector.tensor_mul(t1[:], p_x[:], cosd[:, cols])
                t2 = p2sb.tile([128, SB], BF16, tag="t2")
                nc.vector.tensor_mul(t2[:], p_r[:], sind[:, cols])
                nc.vector.tensor_add(kpd[:, cols], t1[:], t2[:])

                for t4 in range(SB // 128):
                    sl = slice(t4 * 128, (t4 + 1) * 128)
                    p_v = pp2v.tile([128, HPC * VD], F32, tag="vv")
                    vterms = [(xh[b], cvth), (xl[b], cvth), (xh[b], cvtl)]
                    for t, (xx, ww) in enumerate(vterms):
                        for j in range(NJ):
                            nc.tensor.matmul(
                                p_v[:], xx[:, 2 * j:2 * j + 2, sl],
                                ww[:, 2 * j:2 * j + 2, :],
                                start=(t == 0 and j == 0),
                                stop=(t == 2 and j == NJ - 1), perf_mode=DR)
                    nc.any.tensor_copy(vst[:, b * (SB // 128) + t4, :], p_v[:])

            # block 0 first: its inputs lead the DMA queue
            block_proj(0)

            # ---- phase 1 (stats) in the middle; collective hides ----
            with tc.tile_pool(name="pstats", bufs=1) as pst, \
                 tc.tile_pool(name="p1sb", bufs=3) as p1sb, \
                 tc.tile_pool(name="p1st", bufs=1) as p1st, \
                 tc.tile_pool(name="pp1", bufs=2, space="PSUM") as pp1, \
                 tc.tile_pool(name="pp1s", bufs=1, space="PSUM") as pp1s:
                xs = pst.tile([128, KT, SEQB], F8)
                nc.sync.dma_start(out=xs, in_=d["xs"])
                wqa = pst.tile([128, KT, Q_LORA], F8)
                for ch in range(3):
                    cs = slice(ch * 512, (ch + 1) * 512)
                    nc.sync.dma_start(out=wqa[:, :, cs],
                                      in_=d["wqa"][:, :, cs])
                wkvac = pst.tile([128, KT, KV_LORA], F8)
                nc.sync.dma_start(out=wkvac, in_=d["wkvac"])

                p_qs = pp1s.tile([1, SEQB], F32, tag="stat", name="p_qs")
                for m in range(QLT):
                    p_a = pp1.tile([128, SEQB], F32, tag="acc")
                    for j in range(NJ):
                        nc.tensor.matmul(p_a[:], wqa[:, 2 * j:2 * j + 2,
                                                     m * 128:(m + 1) * 128],
                                         xs[:, 2 * j:2 * j + 2, :],
                                         start=(j == 0), stop=(j == NJ - 1),
                                         perf_mode=DR)
                    ql = p1sb.tile([128, SEQB], BF16, tag="ql")
                    nc.any.tensor_copy(ql[:], p_a[:])
                    sq = p1sb.tile([128, SEQB], BF16, tag="sq")
                    nc.vector.tensor_mul(sq[:], ql[:], ql[:])
                    nc.tensor.matmul(p_qs[:], ones_b[:, 0:1], sq[:],
                                     start=(m == 0), stop=(m == QLT - 1))
                qs_s = p1st.tile([1, SEQB], F32, tag="s1")
                nc.scalar.activation(qs_s[:], p_qs[:], Sqrt,
                                     scale=1.0 / (Q_LORA * WS * WS), bias=eps1[:])
                invq_s = p1st.tile([1, SEQB], F32R, tag="s2")
                nc.vector.reciprocal(invq_s[:], qs_s[:])
                nc.gpsimd.dma_start(out=cc_in[0:1, :], in_=invq_s[:])

                p_cs = pp1s.tile([1, SEQB], F32, tag="stat", name="p_cs")
                for m in range(CT):
                    p_a = pp1.tile([128, SEQB], F32, tag="acc")
                    for j in range(NJ):
                        nc.tensor.matmul(p_a[:], wkvac[:, 2 * j:2 * j + 2,
                                                       m * 128:(m + 1) * 128],
                                         xs[:, 2 * j:2 * j + 2, :],
                                         start=(j == 0), stop=(j == NJ - 1),
                                         perf_mode=DR)
                    cl = p1sb.tile([128, SEQB], BF16, tag="ql")
                    nc.any.tensor_copy(cl[:], p_a[:])
                    sq = p1sb.tile([128, SEQB], BF16, tag="sq")
                    nc.vector.tensor_mul(sq[:], cl[:], cl[:])
                    nc.tensor.matmul(p_cs[:], ones_b[:, 0:1], sq[:],
                                     start=(m == 0), stop=(m == CT - 1))
                cs_s = p1st.tile([1, SEQB], F32, tag="s3")
                nc.scalar.activation(cs_s[:], p_cs[:], Sqrt,
                                     scale=1.0 / (KV_LORA * WS * WS), bias=eps1[:])
                invc_s = p1st.tile([1, SEQB], F32R, tag="s4")
                nc.vector.reciprocal(invc_s[:], cs_s[:])
                nc.gpsimd.dma_start(out=cc_in[1:2, :], in_=invc_s[:])

                nc.gpsimd.collective_compute(
                    "AllGather", mybir.AluOpType.bypass,
                    replica_groups=[list(range(NCORES))],
                    ins=[cc_in[:].opt()], outs=[cc_out[:].opt()])
                nc.sync.dma_start(out=invq_row, in_=cc_out[:, 0, :])
                nc.sync.dma_start(out=invc_row, in_=cc_out[:, 1, :])
                for g in range(NCORES):
                    nc.gpsimd.dma_start(
                        out=invc_col[:, g, :],
                        in_=cc_out[g, 1, :].rearrange("(t p) -> p t", p=128))

            # prefetch blocks 1-2 x tiles, then late-need constants
            for bb in (1, 2):
                cols = bass.ts(bb, SB)
                xh[bb] = pxb.tile([128, KT, SB], F8, tag="xh", name=f"xh{bb}")
                nc.sync.dma_start(out=xh[bb], in_=d["xh"][:, :, cols])
                xl[bb] = pxb.tile([128, KT, SB], F8, tag="xl", name=f"xl{bb}")
                nc.sync.dma_start(out=xl[bb], in_=d["xl"][:, :, cols])
            ones_r = pc.tile([128, 128], F32R)
            nc.sync.dma_start(out=ones_r, in_=d["onesr"])
            msk = pc.tile([128, 4, SB], BF16)
            nc.sync.dma_start(out=msk, in_=d["msk"])
            wo = pc.tile([128, HPC, HIDDEN], BF16)
            nc.sync.dma_start(out=wo, in_=d["wo"])

            def scale_block(b):
                # inv_rms application for block b (needs the AllGather)
                cols = bass.ts(b, SB)
                p_bq = pp2.tile([128, SB], F32, tag="mm")
                nc.tensor.matmul(p_bq[:], ones_r[0:1, :],
                                 invq_row[0:1, 2 * b:2 * b + 2, :],
                                 start=True, stop=True)
                nc.any.tensor_copy(invqb[:, cols], p_bq[:])
                p_bc = pp2.tile([128, SB], F32, tag="mm")
                nc.tensor.matmul(p_bc[:], ones_r[0:1, :],
                                 invc_row[0:1, 2 * b:2 * b + 2, :],
                                 start=True, stop=True)
                nc.any.tensor_copy(invcb[:, cols], p_bc[:])
                nc.vector.tensor_mul(qn[0][:, cols], qn[0][:, cols], invqb[:, cols])
                nc.vector.tensor_mul(qn[1][:, cols], qn[1][:, cols], invqb[:, cols])
                nc.vector.tensor_mul(qpe[:, cols], qpe[:, cols], invqb[:, cols])
                nc.vector.tensor_mul(kn[0][:, cols], kn[0][:, cols], invcb[:, cols])
                nc.vector.tensor_mul(kn[1][:, cols], kn[1][:, cols], invcb[:, cols])
                for st in range(4 * b, 4 * (b + 1)):
                    nc.vector.tensor_scalar_mul(
                        vst[:, st, :], vst[:, st, :],
                        invc_col[:, st // 2, st % 2:st % 2 + 1])

            block_proj(1)
            scale_block(0)
            block_proj(2)
            scale_block(1)
            block_proj(3)
            scale_block(2)
            scale_block(3)

        # ------- phase 3: attention per (block, head) + fused o_proj -------
        # q and k each carry the 2^5 weight pre-scale; fold 2^-10 into Exp.
        ESCALE = SCALE / (WS * WS)
        with tc.tile_pool(name="pexp", bufs=3) as pexp, \
             tc.tile_pool(name="pes", bufs=2) as pes, \
             tc.tile_pool(name="pbn", bufs=2) as pbn, \
             tc.tile_pool(name="pout", bufs=4) as pout, \
             tc.tile_pool(name="ppS", bufs=2, space="PSUM") as ppS, \
             tc.tile_pool(name="ppO", bufs=2, space="PSUM") as ppO, \
             tc.tile_pool(name="ppD", bufs=1, space="PSUM") as ppD, \
             tc.tile_pool(name="ppB", bufs=1, space="PSUM") as ppB, \
             tc.tile_pool(name="ppC", bufs=2, space="PSUM") as ppC:
            for qb in range(NSB):
                qcols = bass.ts(qb, SB)
                nk = 4 * (qb + 1)
                for h in range(HPC):
                    hp = slice(64 * h, 64 * h + 64)
                    p_o = ppO.tile([128, SB], F32, tag="o")
                    esum = [pes.tile([128, SB], F32R, tag=f"es{par}",
                                     name=f"es{par}") for par in range(2)]
                    for ik in range(nk):
                        kc = slice(ik * 128, (ik + 1) * 128)
                        r = ik - 4 * qb
                        # diagonal tiles: only queries >= 128r can attend this
                        # k-tile; trim the matmul/exp width (qb>0 so the
                        # first two chain tiles stay full width)
                        lo = 128 * r if (r >= 1 and qb > 0) else 0
                        qsub = slice(qb * SB + lo, (qb + 1) * SB)
                        sub = slice(lo, SB)
                        p_s = ppS.tile([128, SB], F32, tag="s")
                        nc.tensor.matmul(p_s[:, sub], kn[h][:, kc],
                                         qn[h][:, qsub], start=True, stop=False)
                        nc.tensor.matmul(p_s[:, sub], kpd[hp, kc],
                                         qpe[hp, qsub], start=False, stop=True)
                        e = pexp.tile([128, SB], BF16, tag="e")
                        nc.scalar.activation(e[:, sub], p_s[:, sub], Exp,
                                             scale=ESCALE)
                        if r >= 0:
                            nc.vector.tensor_mul(e[:, sub], e[:, sub],
                                                 msk[:, r, sub])
                        nc.tensor.matmul(p_o[:, sub],
                                         vst[:, ik, h * VD:(h + 1) * VD],
                                         e[:, sub], start=(ik == 0),
                                         stop=(ik == nk - 1))
                        es = esum[ik % 2]
                        if ik < 2:
                            nc.vector.tensor_copy(es[:], e[:])
                        else:
                            nc.vector.tensor_add(es[:, sub], es[:, sub],
                                                 e[:, sub])
                    p_d = ppD.tile([1, SB], F32, tag="d")
                    nc.tensor.matmul(p_d[:], ones_r[:, 0:1], esum[0][:],
                                     start=True, stop=False)
                    nc.tensor.matmul(p_d[:], ones_r[:, 0:1], esum[1][:],
                                     start=False, stop=True)
                    rec_s = pbn.tile([1, SB], F32R, tag="rec")
                    nc.vector.reciprocal(rec_s[:], p_d[:])
                    p_bc = ppB.tile([128, SB], F32, tag="bc")
                    nc.tensor.matmul(p_bc[:], ones_r[0:1, :], rec_s[:],
                                     start=True, stop=True)
                    recb = pbn.tile([128, SB], BF16, tag="recb")
                    nc.any.tensor_copy(recb[:], p_bc[:])
                    nc.vector.tensor_mul(ao[h][:, qcols], p_o[:], recb[:])
                for st in range(qb * (SB // 128), (qb + 1) * (SB // 128)):
                    sc = slice(st * 128, (st + 1) * 128)
                    ot = pout.tile([128, HIDDEN], BF16, tag="ot")
                    for nb in range(HIDDEN // SB):
                        ncols = bass.ts(nb, SB)
                        p_c = ppC.tile([128, SB], F32, tag="c")
                        for h in range(HPC):
                            nc.tensor.matmul(p_c[:], ao[h][:, sc], wo[:, h, ncols],
                                             start=(h == 0), stop=(h == HPC - 1))
                        nc.any.tensor_copy(ot[:, ncols], p_c[:])
                    nc.sync.dma_start(out=d["out"][sc, :], in_=ot[:])


def _host_constants():
    inv_freq = 1.0 / (ROPE_THETA ** (np.arange(0, ROPE, dtype=np.float32)[0::2] / ROPE))
    t = np.arange(S, dtype=np.float32)
    freqs = np.outer(t, inv_freq)
    emb = np.concatenate([freqs, freqs], -1)          # [S, 64]
    cos, sin = np.cos(emb), np.sin(emb)
    cosd = np.concatenate([cos.T, cos.T], 0).astype(np.float32)   # [128, S]
    sind = np.concatenate([sin.T, sin.T], 0).astype(np.float32)

    msk = np.zeros((128, 4, SB), np.float32)
    for r in range(4):
        for p in range(128):
            k_idx = p + 128 * r
            if k_idx < SB:
                msk[p, r, k_idx:] = 1.0               # keep where k <= q

    Q = np.zeros((64, 64), np.float32)
    for i in range(32):
        Q[i, i + 32] = -1.0
        Q[i + 32, i] = 1.0
    P = np.zeros((128, 128), np.float32)
    P[:64, :64] = Q
    P[64:, 64:] = Q
    rotq = P.T.copy()
    D = np.concatenate([np.eye(64, dtype=np.float32)] * 2, 0)     # [128, 64]
    dupx = D.T.copy()                                  # [64, 128]
    duprot = np.concatenate([Q, Q], 0).T.copy()        # [64, 128]
    return cosd, sind, msk, rotq, dupx, duprot


def kernel(hidden_states, w_q_a, q_a_weight, w_q_b, w_kv_a, kv_a_weight,
           w_kv_b, w_o):
    global LAST_RESULT
    import ml_dtypes
    bf16 = ml_dtypes.bfloat16
    f8 = ml_dtypes.float8_e4m3
    if "nc" not in _CACHE:
        _CACHE["nc"] = _build_program()
    nc = _CACHE["nc"]

    def b(a):
        return np.ascontiguousarray(np.asarray(a, np.float32).astype(bf16))

    def e8(a):
        return np.ascontiguousarray(np.asarray(a, np.float32).astype(f8))

    def pre(a):
        """[HIDDEN, F] -> [128, KT, F] partition-major prearrangement."""
        a = np.asarray(a)
        t = a.shape[0] // 128
        return np.ascontiguousarray(
            a.reshape(t, 128, a.shape[1]).transpose(1, 0, 2))

    def split8(a):
        """hi/lo fp8 split of an already-scaled array, prearranged."""
        a = np.asarray(a, np.float32)
        hi = a.astype(f8)
        lo = (a - hi.astype(np.float32)).astype(f8)
        return pre(hi), pre(lo)

    x = np.asarray(hidden_states, np.float32)[0]       # [S, 2048]
    xt = x.T                                           # [2048, S]
    wqa = np.asarray(w_q_a, np.float32)                # [1536, 2048]
    wkva = np.asarray(w_kv_a, np.float32)              # [576, 2048]
    wqb_eff = np.asarray(w_q_b, np.float32) * np.asarray(q_a_weight, np.float32)[None, :]
    wkvb_eff = np.asarray(w_kv_b, np.float32) * np.asarray(kv_a_weight, np.float32)[None, :]
    won = np.asarray(w_o, np.float32)

    cosd, sind, msk, rotq, dupx, duprot = _host_constants()
    onesm = np.ones((128, 128), np.float32)
    xh8, xl8 = split8(xt)
    wpeh8, wpel8 = split8(WS * wkva[KV_LORA:].T)
    shared = {"onesb": b(onesm), "onesr": onesm,
              "xh": xh8, "xl": xl8,
              "wqa_t": pre(e8(WS * wqa.T)), "wkvac_t": pre(e8(WS * wkva[:KV_LORA].T)),
              "wpeh": wpeh8, "wpel": wpel8,
              "cosd": b(cosd), "sind": b(sind), "mask": b(msk),
              "rotq": b(rotq), "dupx": b(dupx), "duprot": b(duprot)}

    in_maps = []
    for c in range(NCORES):
        h0, h1 = HPC * c, HPC * c + 1
        rows_q = np.concatenate(
            [wqb_eff[h0 * QD:h0 * QD + NOPE],
             wqb_eff[h1 * QD:h1 * QD + NOPE],
             wqb_eff[h0 * QD + NOPE:(h0 + 1) * QD],
             wqb_eff[h1 * QD + NOPE:(h1 + 1) * QD]], 0)          # [384, 1536]
        B = rows_q @ wqa                                          # [384, 2048]
        rows_k = np.concatenate(
            [wkvb_eff[h * (NOPE + VD):h * (NOPE + VD) + NOPE] for h in (h0, h1)], 0)
        Ck = rows_k @ wkva[:KV_LORA]
        rows_v = np.concatenate(
            [wkvb_eff[h * (NOPE + VD) + NOPE:(h + 1) * (NOPE + VD)]
             for h in (h0, h1)], 0)
        Cv = rows_v @ wkva[:KV_LORA]
        # v path carries the 2^5 Cv pre-scale: fold 2^-5 into w_o
        wo_t = np.concatenate(
            [won[:, h * VD:(h + 1) * VD] for h in (h0, h1)], 1).T / WS
        bth8, btl8 = split8(WS * B.T)
        ckth8, cktl8 = split8(WS * Ck.T)
        cvth8, cvtl8 = split8(WS * Cv.T)
        im = dict(shared)
        im.update({"bth": bth8, "btl": btl8, "ckth": ckth8, "cktl": cktl8,
                   "cvth": cvth8, "cvtl": cvtl8,
                   "wo_t": pre(b(wo_t)).reshape(128, HPC, HIDDEN),
                   "xs": pre(e8(xt[:, SEQB * c:SEQB * (c + 1)]))})
        in_maps.append(im)

    res = run_bass_kernel_spmd(nc, in_maps, list(range(NCORES)))
    LAST_RESULT = res
    out = np.zeros((S, HIDDEN), np.float32)
    for c in range(NCORES):
        out += res.results[c]["out"].astype(np.float32)
    return out.reshape(1, S, HIDDEN)


# revision 3
# speedup vs baseline: 1.0120x; 1.0120x over previous
"""MLA forward on 8 trn2 cores — absorbed-weight tensor-parallel version.

Key algebraic move: RMSNorm(z) = z * inv_rms(z) * w with inv_rms a per-token
scalar, so the LoRA up-projections absorb the down-projections on the host:
  q   = (x @ (Wqb diag(w) Wqa)^T) * inv_rms(x Wqa^T)
  k,v = (x @ (Wkvb diag(w) Wkva_c)^T) * inv_rms(x Wkva_c^T)
Each core only computes the absorbed GEMMs for its 2 heads instead of the
replicated LoRA-A GEMMs. The per-token inv_rms scalars still need the full
latent rows; those are computed seq-sharded (256 tokens per core) and
exchanged with a single 16 KB AllGather whose latency hides under the
absorbed GEMMs. inv_rms scaling is deferred until after the gather (rope
commutes with per-token scaling).

Precision plan (tolerance 2e-2, validated offline in fp8_sim.py):
- stats GEMMs: fp8e4 DoubleRow, 1-term (inv_rms only needs ~0.2%)
- absorbed GEMMs: fp8e4 DoubleRow, 3-term compensated
  (x8@w_hi + r8@w_hi + x8@w_lo), host-prepared splits
- attention + o_proj: bf16 operands, fp32 PSUM
Weights are pre-scaled by 2^5 to center e4m3; the factor is folded into the
Sqrt activation scale (stats), the Exp scale (q·k carries 2^10), and into
w_o on the host (v path). Softmax denominators accumulate on DVE. o_proj
partials are summed on the host across cores (bf16 partial writes).
"""
import numpy as np

import concourse.bass as bass
import concourse.tile as tile
from concourse import bacc, mybir
from concourse.bass_utils import run_bass_kernel_spmd

F32 = mybir.dt.float32
F32R = mybir.dt.float32r
BF16 = mybir.dt.bfloat16
F8 = mybir.dt.float8e4

HIDDEN = 2048
S = 2048
NUM_HEADS = 16
Q_LORA = 1536
KV_LORA = 512
NOPE = 128
ROPE = 64
VD = 128
QD = NOPE + ROPE            # 192
SCALE = QD ** -0.5
EPS = 1e-6
ROPE_THETA = 10000.0

NCORES = 8
HPC = NUM_HEADS // NCORES   # 2
SB = 512
NSB = S // SB               # 4
KT = HIDDEN // 128          # 16
NJ = KT // 2                # 8 DoubleRow k-pairs
SEQB = S // NCORES          # 256 stats tokens per core
QMT = (HPC * QD) // 128     # 3
KMT = (HPC * NOPE) // 128   # 2
WS = 32.0                   # 2^5 weight pre-scale for e4m3

_CACHE = {}
LAST_RESULT = None


def _build_program():
    nc = bacc.Bacc("TRN2", target_bir_lowering=False, debug=False,
                   num_devices=NCORES)
    dt = nc.dram_tensor
    d = {
        "xs": dt("xs", [128, KT, SEQB], F8, kind="ExternalInput").ap(),
        "wqa": dt("wqa_t", [128, KT, Q_LORA], F8, kind="ExternalInput").ap(),
        "wkvac": dt("wkvac_t", [128, KT, KV_LORA], F8, kind="ExternalInput").ap(),
        "xh": dt("xh", [128, KT, S], F8, kind="ExternalInput").ap(),
        "xl": dt("xl", [128, KT, S], F8, kind="ExternalInput").ap(),
        "bth": dt("bth", [128, KT, HPC * QD], F8, kind="ExternalInput").ap(),
        "btl": dt("btl", [128, KT, HPC * QD], F8, kind="ExternalInput").ap(),
        "ckth": dt("ckth", [128, KT, HPC * NOPE], F8, kind="ExternalInput").ap(),
        "cktl": dt("cktl", [128, KT, HPC * NOPE], F8, kind="ExternalInput").ap(),
        "cvth": dt("cvth", [128, KT, HPC * VD], F8, kind="ExternalInput").ap(),
        "cvtl": dt("cvtl", [128, KT, HPC * VD], F8, kind="ExternalInput").ap(),
        "wpeh": dt("wpeh", [128, KT, ROPE], F8, kind="ExternalInput").ap(),
        "wpel": dt("wpel", [128, KT, ROPE], F8, kind="ExternalInput").ap(),
        "wo": dt("wo_t", [128, HPC, HIDDEN], BF16, kind="ExternalInput").ap(),
        "cos": dt("cosd", [128, S], BF16, kind="ExternalInput").ap(),
        "sin": dt("sind", [128, S], BF16, kind="ExternalInput").ap(),
        "msk": dt("mask", [128, 4, SB], BF16, kind="ExternalInput").ap(),
        "onesb": dt("onesb", [128, 128], BF16, kind="ExternalInput").ap(),
        "onesr": dt("onesr", [128, 128], F32R, kind="ExternalInput").ap(),
        "rotq": dt("rotq", [128, 128], BF16, kind="ExternalInput").ap(),
        "dupx": dt("dupx", [64, 128], BF16, kind="ExternalInput").ap(),
        "duprot": dt("duprot", [64, 128], BF16, kind="ExternalInput").ap(),
        "out": dt("out", [S, HIDDEN], BF16, kind="ExternalOutput").ap(),
    }
    with tile.TileContext(nc) as tc:
        _mla(tc, d)
    nc.compile()
    return nc


def _rearr(ap):
    return ap.rearrange("(t p) f -> p t f", p=128)


def _mla(tc, d):
    nc = tc.nc
    Exp = mybir.ActivationFunctionType.Exp
    Sqrt = mybir.ActivationFunctionType.Sqrt
    DR = mybir.MatmulPerfMode.DoubleRow
    QLT = Q_LORA // 128     # 12
    CT = KV_LORA // 128     # 4

    with nc.allow_low_precision(reason="fp8/bf16 matmul pipeline with fp32 "
                                "accumulation; tolerance is 2e-2"), \
         tc.tile_pool(name="pxb", bufs=2) as pxb, \
         tc.tile_pool(name="pconst", bufs=1) as pc, \
         tc.tile_pool(name="pqkv", bufs=1) as pqkv, \
         tc.tile_pool(name="pdram", bufs=1, space="DRAM") as pdram:
        # ---- DMAs in PE-consumption order: block-0 inputs lead ----
        xh = {}
        xl = {}
        xh[0] = pxb.tile([128, KT, SB], F8, tag="xh", name="xh0")
        bth = pc.tile([128, KT, HPC * QD], F8)
        xl[0] = pxb.tile([128, KT, SB], F8, tag="xl", name="xl0")
        btl = pc.tile([128, KT, HPC * QD], F8)
        for kk in (slice(0, 8), slice(8, KT)):
            nc.sync.dma_start(out=xh[0][:, kk, :], in_=d["xh"][:, kk, 0:SB])
            nc.sync.dma_start(out=bth[:, kk, :], in_=d["bth"][:, kk, :])
            nc.sync.dma_start(out=xl[0][:, kk, :], in_=d["xl"][:, kk, 0:SB])
            nc.sync.dma_start(out=btl[:, kk, :], in_=d["btl"][:, kk, :])
        ones_b = pc.tile([128, 128], BF16)
        nc.sync.dma_start(out=ones_b, in_=d["onesb"])
        ckth = pc.tile([128, KT, HPC * NOPE], F8)
        nc.sync.dma_start(out=ckth, in_=d["ckth"])
        cktl = pc.tile([128, KT, HPC * NOPE], F8)
        nc.sync.dma_start(out=cktl, in_=d["cktl"])
        wpeh = pc.tile([128, KT, ROPE], F8)
        nc.sync.dma_start(out=wpeh, in_=d["wpeh"])
        wpel = pc.tile([128, KT, ROPE], F8)
        nc.sync.dma_start(out=wpel, in_=d["wpel"])
        cvth = pc.tile([128, KT, HPC * VD], F8)
        nc.sync.dma_start(out=cvth, in_=d["cvth"])
        cvtl = pc.tile([128, KT, HPC * VD], F8)
        nc.sync.dma_start(out=cvtl, in_=d["cvtl"])
        rotq = pc.tile([128, 128], BF16)
        nc.sync.dma_start(out=rotq, in_=d["rotq"])
        dupx = pc.tile([64, 128], BF16)
        nc.sync.dma_start(out=dupx, in_=d["dupx"])
        duprot = pc.tile([64, 128], BF16)
        nc.sync.dma_start(out=duprot, in_=d["duprot"])
        cosd = pc.tile([128, S], BF16)
        nc.sync.dma_start(out=cosd, in_=d["cos"])
        sind = pc.tile([128, S], BF16)
        nc.sync.dma_start(out=sind, in_=d["sin"])
        eps1 = pc.tile([1, 1], F32)
        nc.vector.memset(eps1, EPS)

        # ---- persistent per-head tensors ----
        qn = [pqkv.tile([128, S], BF16, tag=f"qn{h}", name=f"qn{h}")
              for h in range(HPC)]
        kn = [pqkv.tile([128, S], BF16, tag=f"kn{h}", name=f"kn{h}")
              for h in range(HPC)]
        qpe = pqkv.tile([128, S], BF16, tag="qpe")
        kpd = pqkv.tile([128, S], BF16, tag="kpd")
        vst = pqkv.tile([128, S // 128, HPC * VD], BF16, tag="vst")
        ao = [pqkv.tile([128, S], BF16, tag=f"ao{h}", name=f"ao{h}")
              for h in range(HPC)]
        invqb = pqkv.tile([128, S], BF16, tag="invqb")
        invcb = pqkv.tile([128, S], BF16, tag="invcb")
        invq_row = pqkv.tile([1, NCORES, SEQB], F32R, tag="invq_row")
        invc_row = pqkv.tile([1, NCORES, SEQB], F32R, tag="invc_row")
        invc_col = pqkv.tile([128, NCORES, SEQB // 128], F32, tag="invc_col")

        cc_in = pdram.tile([2, SEQB], F32R)
        cc_out = pdram.tile([NCORES, 2, SEQB], F32R)

        with tc.tile_pool(name="p2sb", bufs=2) as p2sb, \
             tc.tile_pool(name="pp2", bufs=3, space="PSUM") as pp2, \
             tc.tile_pool(name="pp2v", bufs=2, space="PSUM") as pp2v:

            def absorbed(p, wh, wl, ws, xhb, xlb, nt=3):
                """nt-term compensated fp8 DoubleRow accumulation into psum.
                3-term: x8@w_hi + r8@w_hi + x8@w_lo; 2-term drops w_lo."""
                terms = [(xhb, wh), (xlb, wh), (xhb, wl)][:nt]
                for t, (xx, ww) in enumerate(terms):
                    for j in range(NJ):
                        nc.tensor.matmul(
                            p, ww[:, 2 * j:2 * j + 2, ws],
                            xx[:, 2 * j:2 * j + 2, :],
                            start=(t == 0 and j == 0),
                            stop=(t == nt - 1 and j == NJ - 1), perf_mode=DR)

            def block_proj(b):
                cols = bass.ts(b, SB)
                if b >= 3:
                    xh[b] = pxb.tile([128, KT, SB], F8, tag="xh", name=f"xh{b}")
                    nc.sync.dma_start(out=xh[b], in_=d["xh"][:, :, cols])
                    xl[b] = pxb.tile([128, KT, SB], F8, tag="xl", name=f"xl{b}")
                    nc.sync.dma_start(out=xl[b], in_=d["xl"][:, :, cols])

                qdst = [qn[0], qn[1]]
                for mt in range(QMT):
                    p_q = pp2.tile([128, SB], F32, tag="mm")
                    absorbed(p_q[:], bth, btl,
                             slice(mt * 128, (mt + 1) * 128), xh[b], xl[b])
                    if mt < 2:
                        nc.any.tensor_copy(qdst[mt][:, cols], p_q[:])
                    else:
                        qpe_u = p2sb.tile([128, SB], BF16, tag="t0")
                        nc.any.tensor_copy(qpe_u[:], p_q[:])
                        p_rq = pp2.tile([128, SB], F32, tag="mm")
                        nc.tensor.matmul(p_rq[:], rotq[:], qpe_u[:],
                                         start=True, stop=True)
                        t1 = p2sb.tile([128, SB], BF16, tag="t1")
                        nc.vector.tensor_mul(t1[:], qpe_u[:], cosd[:, cols])
                        t2 = p2sb.tile([128, SB], BF16, tag="t2")
                        nc.vector.tensor_mul(t2[:], p_rq[:], sind[:, cols])
                        nc.vector.tensor_add(qpe[:, cols], t1[:], t2[:])

                for mt in range(KMT):
                    p_k = pp2.tile([128, SB], F32, tag="mm")
                    absorbed(p_k[:], ckth, cktl,
                             slice(mt * 128, (mt + 1) * 128), xh[b], xl[b],
                             nt=2)
                    nc.any.tensor_copy(kn[mt][:, cols], p_k[:])

                p_pe = pp2.tile([128, SB], F32, tag="mm")
                absorbed(p_pe[:ROPE, :], wpeh, wpel, slice(0, ROPE),
                         xh[b], xl[b], nt=2)
                kpe_u = p2sb.tile([ROPE, SB], BF16, tag="t3")
                nc.any.tensor_copy(kpe_u[:], p_pe[:ROPE, :])
                p_x = pp2.tile([128, SB], F32, tag="mm")
                nc.tensor.matmul(p_x[:], dupx[:], kpe_u[:], start=True, stop=True)
                p_r = pp2.tile([128, SB], F32, tag="mm")
                nc.tensor.matmul(p_r[:], duprot[:], kpe_u[:], start=True, stop=True)
                t1 = p2sb.tile([128, SB], BF16, tag="t1")
                nc.vector.tensor_mul(t1[:], p_x[:], cosd[:, cols])
                t2 = p2sb.tile([128, SB], BF16, tag="t2")
                nc.vector.tensor_mul(t2[:], p_r[:], sind[:, cols])
                nc.vector.tensor_add(kpd[:, cols], t1[:], t2[:])

                for t4 in range(SB // 128):
                    sl = slice(t4 * 128, (t4 + 1) * 128)
                    p_v = pp2v.tile([128, HPC * VD], F32, tag="vv")
                    vterms = [(xh[b], cvth), (xl[b], cvth), (xh[b], cvtl)]
                    for t, (xx, ww) in enumerate(vterms):
                        for j in range(NJ):
                            nc.tensor.matmul(
                                p_v[:], xx[:, 2 * j:2 * j + 2, sl],
                                ww[:, 2 * j:2 * j + 2, :],
                                start=(t == 0 and j == 0),
                                stop=(t == 2 and j == NJ - 1), perf_mode=DR)
                    nc.any.tensor_copy(vst[:, b * (SB // 128) + t4, :], p_v[:])

            # block 0 first: its inputs lead the DMA queue
            block_proj(0)

            # ---- phase 1 (stats) in the middle; collective hides ----
            with tc.tile_pool(name="pstats", bufs=1) as pst, \
                 tc.tile_pool(name="p1sb", bufs=3) as p1sb, \
                 tc.tile_pool(name="p1st", bufs=1) as p1st, \
                 tc.tile_pool(name="pp1", bufs=2, space="PSUM") as pp1, \
                 tc.tile_pool(name="pp1s", bufs=1, space="PSUM") as pp1s:
                xs = pst.tile([128, KT, SEQB], F8)
                nc.sync.dma_start(out=xs, in_=d["xs"])
                wqa = pst.tile([128, KT, Q_LORA], F8)
                for ch in range(3):
                    cs = slice(ch * 512, (ch + 1) * 512)
                    nc.sync.dma_start(out=wqa[:, :, cs],
                                      in_=d["wqa"][:, :, cs])
                wkvac = pst.tile([128, KT, KV_LORA], F8)
                nc.sync.dma_start(out=wkvac, in_=d["wkvac"])

                p_qs = pp1s.tile([1, SEQB], F32, tag="stat", name="p_qs")
                for m in range(QLT):
                    p_a = pp1.tile([128, SEQB], F32, tag="acc")
                    for j in range(NJ):
                        nc.tensor.matmul(p_a[:], wqa[:, 2 * j:2 * j + 2,
                                                     m * 128:(m + 1) * 128],
                                         xs[:, 2 * j:2 * j + 2, :],
                                         start=(j == 0), stop=(j == NJ - 1),
                                         perf_mode=DR)
                    ql = p1sb.tile([128, SEQB], BF16, tag="ql")
                    nc.any.tensor_copy(ql[:], p_a[:])
                    sq = p1sb.tile([128, SEQB], BF16, tag="sq")
                    nc.vector.tensor_mul(sq[:], ql[:], ql[:])
                    nc.tensor.matmul(p_qs[:], ones_b[:, 0:1], sq[:],
                                     start=(m == 0), stop=(m == QLT - 1))
                qs_s = p1st.tile([1, SEQB], F32, tag="s1")
                nc.scalar.activation(qs_s[:], p_qs[:], Sqrt,
                                     scale=1.0 / (Q_LORA * WS * WS), bias=eps1[:])
                invq_s = p1st.tile([1, SEQB], F32R, tag="s2")
                nc.vector.reciprocal(invq_s[:], qs_s[:])
                nc.gpsimd.dma_start(out=cc_in[0:1, :], in_=invq_s[:])

                p_cs = pp1s.tile([1, SEQB], F32, tag="stat", name="p_cs")
                for m in range(CT):
                    p_a = pp1.tile([128, SEQB], F32, tag="acc")
                    for j in range(NJ):
                        nc.tensor.matmul(p_a[:], wkvac[:, 2 * j:2 * j + 2,
                                                       m * 128:(m + 1) * 128],
                                         xs[:, 2 * j:2 * j + 2, :],
                                         start=(j == 0), stop=(j == NJ - 1),
                                         perf_mode=DR)
                    cl = p1sb.tile([128, SEQB], BF16, tag="ql")
                    nc.any.tensor_copy(cl[:], p_a[:])
                    sq = p1sb.tile([128, SEQB], BF16, tag="sq")
                    nc.vector.tensor_mul(sq[:], cl[:], cl[:])
                    nc.tensor.matmul(p_cs[:], ones_b[:, 0:1], sq[:],
                                     start=(m == 0), stop=(m == CT - 1))
                cs_s = p1st.tile([1, SEQB], F32, tag="s3")
                nc.scalar.activation(cs_s[:], p_cs[:], Sqrt,
                                     scale=1.0 / (KV_LORA * WS * WS), bias=eps1[:])
                invc_s = p1st.tile([1, SEQB], F32R, tag="s4")
                nc.vector.reciprocal(invc_s[:], cs_s[:])
                nc.gpsimd.dma_start(out=cc_in[1:2, :], in_=invc_s[:])

                nc.gpsimd.collective_compute(
                    "AllGather", mybir.AluOpType.bypass,
                    replica_groups=[list(range(NCORES))],
                    ins=[cc_in[:].opt()], outs=[cc_out[:].opt()])
                nc.sync.dma_start(out=invq_row, in_=cc_out[:, 0, :])
                nc.sync.dma_start(out=invc_row, in_=cc_out[:, 1, :])
                for g in range(NCORES):
                    nc.gpsimd.dma_start(
                        out=invc_col[:, g, :],
                        in_=cc_out[g, 1, :].rearrange("(t p) -> p t", p=128))

            # prefetch blocks 1-2 x tiles, then late-need constants
            for bb in (1, 2):
                cols = bass.ts(bb, SB)
                xh[bb] = pxb.tile([128, KT, SB], F8, tag="xh", name=f"xh{bb}")
                nc.sync.dma_start(out=xh[bb], in_=d["xh"][:, :, cols])
                xl[bb] = pxb.tile([128, KT, SB], F8, tag="xl", name=f"xl{bb}")
                nc.sync.dma_start(out=xl[bb], in_=d["xl"][:, :, cols])
            ones_r = pc.tile([128, 128], F32R)
            nc.sync.dma_start(out=ones_r, in_=d["onesr"])
            msk = pc.tile([128, 4, SB], BF16)
            nc.sync.dma_start(out=msk, in_=d["msk"])
            wo = pc.tile([128, HPC, HIDDEN], BF16)
            nc.sync.dma_start(out=wo, in_=d["wo"])

            def scale_block(b):
                # inv_rms application for block b (needs the AllGather)
                cols = bass.ts(b, SB)
                p_bq = pp2.tile([128, SB], F32, tag="mm")
                nc.tensor.matmul(p_bq[:], ones_r[0:1, :],
                                 invq_row[0:1, 2 * b:2 * b + 2, :],
                                 start=True, stop=True)
                nc.any.tensor_copy(invqb[:, cols], p_bq[:])
                p_bc = pp2.tile([128, SB], F32, tag="mm")
                nc.tensor.matmul(p_bc[:], ones_r[0:1, :],
                                 invc_row[0:1, 2 * b:2 * b + 2, :],
                                 start=True, stop=True)
                nc.any.tensor_copy(invcb[:, cols], p_bc[:])
                nc.vector.tensor_mul(qn[0][:, cols], qn[0][:, cols], invqb[:, cols])
                nc.vector.tensor_mul(qn[1][:, cols], qn[1][:, cols], invqb[:, cols])
                nc.vector.tensor_mul(qpe[:, cols], qpe[:, cols], invqb[:, cols])
                nc.vector.tensor_mul(kn[0][:, cols], kn[0][:, cols], invcb[:, cols])
                nc.vector.tensor_mul(kn[1][:, cols], kn[1][:, cols], invcb[:, cols])
                for st in range(4 * b, 4 * (b + 1)):
                    nc.vector.tensor_scalar_mul(
                        vst[:, st, :], vst[:, st, :],
                        invc_col[:, st // 2, st % 2:st % 2 + 1])

            block_proj(1)
            scale_block(0)
            block_proj(2)
            scale_block(1)
            block_proj(3)
            scale_block(2)
            scale_block(3)

        # ------- phase 3: attention per (block, head) + fused o_proj -------
        # q and k each carry the 2^5 weight pre-scale; fold 2^-10 into Exp.
        ESCALE = SCALE / (WS * WS)
        with tc.tile_pool(name="pexp", bufs=3) as pexp, \
             tc.tile_pool(name="pes", bufs=2) as pes, \
             tc.tile_pool(name="pbn", bufs=2) as pbn, \
             tc.tile_pool(name="pout", bufs=4) as pout, \
             tc.tile_pool(name="ppS", bufs=3, space="PSUM") as ppS, \
             tc.tile_pool(name="ppO", bufs=2, space="PSUM") as ppO, \
             tc.tile_pool(name="ppB", bufs=1, space="PSUM") as ppB, \
             tc.tile_pool(name="ppC", bufs=2, space="PSUM") as ppC:
            for qb in range(NSB):
                qcols = bass.ts(qb, SB)
                nk = 4 * (qb + 1)
                for h in range(HPC):
                    hp = slice(64 * h, 64 * h + 64)
                    p_o = ppO.tile([128, SB], F32, tag="o")
                    esum = [pes.tile([128, SB], F32R, tag=f"es{par}",
                                     name=f"es{par}") for par in range(2)]
                    for ik in range(nk):
                        kc = slice(ik * 128, (ik + 1) * 128)
                        r = ik - 4 * qb
                        # diagonal tiles: only queries >= 128r can attend this
                        # k-tile; trim the matmul/exp width (qb>0 so the
                        # first two chain tiles stay full width)
                        lo = 128 * r if (r >= 1 and qb > 0) else 0
                        qsub = slice(qb * SB + lo, (qb + 1) * SB)
                        sub = slice(lo, SB)
                        p_s = ppS.tile([128, SB], F32, tag="s")
                        nc.tensor.matmul(p_s[:, sub], kn[h][:, kc],
                                         qn[h][:, qsub], start=True, stop=False)
                        nc.tensor.matmul(p_s[:, sub], kpd[hp, kc],
                                         qpe[hp, qsub], start=False, stop=True)
                        e = pexp.tile([128, SB], BF16, tag="e")
                        nc.scalar.activation(e[:, sub], p_s[:, sub], Exp,
                                             scale=ESCALE)
                        if r >= 0:
                            nc.vector.tensor_mul(e[:, sub], e[:, sub],
                                                 msk[:, r, sub])
                        nc.tensor.matmul(p_o[:, sub],
                                         vst[:, ik, h * VD:(h + 1) * VD],
                                         e[:, sub], start=(ik == 0),
                                         stop=(ik == nk - 1))
                        es = esum[ik % 2]
                        if ik < 2:
                            nc.vector.tensor_copy(es[:], e[:])
                        else:
                            nc.vector.tensor_add(es[:, sub], es[:, sub],
                                                 e[:, sub])
                    p_bc = ppB.tile([128, SB], F32, tag="bc")
                    p_d = p_bc[0:1, :]
                    nc.tensor.matmul(p_d, ones_r[:, 0:1], esum[0][:],
                                     start=True, stop=False)
                    nc.tensor.matmul(p_d, ones_r[:, 0:1], esum[1][:],
                                     start=False, stop=True)
                    rec_s = pbn.tile([1, SB], F32R, tag="rec")
                    nc.vector.reciprocal(rec_s[:], p_d)
                    nc.tensor.matmul(p_bc[:], ones_r[0:1, :], rec_s[:],
                                     start=True, stop=True)
                    recb = pbn.tile([128, SB], BF16, tag="recb")
                    nc.any.tensor_copy(recb[:], p_bc[:])
                    nc.vector.tensor_mul(ao[h][:, qcols], p_o[:], recb[:])
                for st in range(qb * (SB // 128), (qb + 1) * (SB // 128)):
                    sc = slice(st * 128, (st + 1) * 128)
                    ot = pout.tile([128, HIDDEN], BF16, tag="ot")
                    for nb in range(HIDDEN // SB):
                        ncols = bass.ts(nb, SB)
                        p_c = ppC.tile([128, SB], F32, tag="c")
                        for h in range(HPC):
                            nc.tensor.matmul(p_c[:], ao[h][:, sc], wo[:, h, ncols],
                                             start=(h == 0), stop=(h == HPC - 1))
                        nc.any.tensor_copy(ot[:, ncols], p_c[:])
                    nc.sync.dma_start(out=d["out"][sc, :], in_=ot[:])


def _host_constants():
    inv_freq = 1.0 / (ROPE_THETA ** (np.arange(0, ROPE, dtype=np.float32)[0::2] / ROPE))
    t = np.arange(S, dtype=np.float32)
    freqs = np.outer(t, inv_freq)
    emb = np.concatenate([freqs, freqs], -1)          # [S, 64]
    cos, sin = np.cos(emb), np.sin(emb)
    cosd = np.concatenate([cos.T, cos.T], 0).astype(np.float32)   # [128, S]
    sind = np.concatenate([sin.T, sin.T], 0).astype(np.float32)

    msk = np.zeros((128, 4, SB), np.float32)
    for r in range(4):
        for p in range(128):
            k_idx = p + 128 * r
            if k_idx < SB:
                msk[p, r, k_idx:] = 1.0               # keep where k <= q

    Q = np.zeros((64, 64), np.float32)
    for i in range(32):
        Q[i, i + 32] = -1.0
        Q[i + 32, i] = 1.0
    P = np.zeros((128, 128), np.float32)
    P[:64, :64] = Q
    P[64:, 64:] = Q
    rotq = P.T.copy()
    D = np.concatenate([np.eye(64, dtype=np.float32)] * 2, 0)     # [128, 64]
    dupx = D.T.copy()                                  # [64, 128]
    duprot = np.concatenate([Q, Q], 0).T.copy()        # [64, 128]
    return cosd, sind, msk, rotq, dupx, duprot


def kernel(hidden_states, w_q_a, q_a_weight, w_q_b, w_kv_a, kv_a_weight,
           w_kv_b, w_o):
    global LAST_RESULT
    import ml_dtypes
    bf16 = ml_dtypes.bfloat16
    f8 = ml_dtypes.float8_e4m3
    if "nc" not in _CACHE:
        _CACHE["nc"] = _build_program()
    nc = _CACHE["nc"]

    def b(a):
        return np.ascontiguousarray(np.asarray(a, np.float32).astype(bf16))

    def e8(a):
        return np.ascontiguousarray(np.asarray(a, np.float32).astype(f8))

    def pre(a):
        """[HIDDEN, F] -> [128, KT, F] partition-major prearrangement."""
        a = np.asarray(a)
        t = a.shape[0] // 128
        return np.ascontiguousarray(
            a.reshape(t, 128, a.shape[1]).transpose(1, 0, 2))

    def split8(a):
        """hi/lo fp8 split of an already-scaled array, prearranged."""
        a = np.asarray(a, np.float32)
        hi = a.astype(f8)
        lo = (a - hi.astype(np.float32)).astype(f8)
        return pre(hi), pre(lo)

    x = np.asarray(hidden_states, np.float32)[0]       # [S, 2048]
    xt = x.T                                           # [2048, S]
    wqa = np.asarray(w_q_a, np.float32)                # [1536, 2048]
    wkva = np.asarray(w_kv_a, np.float32)              # [576, 2048]
    wqb_eff = np.asarray(w_q_b, np.float32) * np.asarray(q_a_weight, np.float32)[None, :]
    wkvb_eff = np.asarray(w_kv_b, np.float32) * np.asarray(kv_a_weight, np.float32)[None, :]
    won = np.asarray(w_o, np.float32)

    cosd, sind, msk, rotq, dupx, duprot = _host_constants()
    onesm = np.ones((128, 128), np.float32)
    xh8, xl8 = split8(xt)
    wpeh8, wpel8 = split8(WS * wkva[KV_LORA:].T)
    shared = {"onesb": b(onesm), "onesr": onesm,
              "xh": xh8, "xl": xl8,
              "wqa_t": pre(e8(WS * wqa.T)), "wkvac_t": pre(e8(WS * wkva[:KV_LORA].T)),
              "wpeh": wpeh8, "wpel": wpel8,
              "cosd": b(cosd), "sind": b(sind), "mask": b(msk),
              "rotq": b(rotq), "dupx": b(dupx), "duprot": b(duprot)}

    in_maps = []
    for c in range(NCORES):
        h0, h1 = HPC * c, HPC * c + 1
        rows_q = np.concatenate(
            [wqb_eff[h0 * QD:h0 * QD + NOPE],
             wqb_eff[h1 * QD:h1 * QD + NOPE],
             wqb_eff[h0 * QD + NOPE:(h0 + 1) * QD],
             wqb_eff[h1 * QD + NOPE:(h1 + 1) * QD]], 0)          # [384, 1536]
        B = rows_q @ wqa                                          # [384, 2048]
        rows_k = np.concatenate(
            [wkvb_eff[h * (NOPE + VD):h * (NOPE + VD) + NOPE] for h in (h0, h1)], 0)
        Ck = rows_k @ wkva[:KV_LORA]
        rows_v = np.concatenate(
            [wkvb_eff[h * (NOPE + VD) + NOPE:(h + 1) * (NOPE + VD)]
             for h in (h0, h1)], 0)
        Cv = rows_v @ wkva[:KV_LORA]
        # v path carries the 2^5 Cv pre-scale: fold 2^-5 into w_o
        wo_t = np.concatenate(
            [won[:, h * VD:(h + 1) * VD] for h in (h0, h1)], 1).T / WS
        bth8, btl8 = split8(WS * B.T)
        ckth8, cktl8 = split8(WS * Ck.T)
        cvth8, cvtl8 = split8(WS * Cv.T)
        im = dict(shared)
        im.update({"bth": bth8, "btl": btl8, "ckth": ckth8, "cktl": cktl8,
                   "cvth": cvth8, "cvtl": cvtl8,
                   "wo_t": pre(b(wo_t)).reshape(128, HPC, HIDDEN),
                   "xs": pre(e8(xt[:, SEQB * c:SEQB * (c + 1)]))})
        in_maps.append(im)

    res = run_bass_kernel_spmd(nc, in_maps, list(range(NCORES)))
    LAST_RESULT = res
    out = np.zeros((S, HIDDEN), np.float32)
    for c in range(NCORES):
        out += res.results[c]["out"].astype(np.float32)
    return out.reshape(1, S, HIDDEN)


# revision 4
# speedup vs baseline: 1.0209x; 1.0088x over previous
"""MLA forward on 8 trn2 cores — absorbed-weight tensor-parallel version.

Key algebraic move: RMSNorm(z) = z * inv_rms(z) * w with inv_rms a per-token
scalar, so the LoRA up-projections absorb the down-projections on the host:
  q   = (x @ (Wqb diag(w) Wqa)^T) * inv_rms(x Wqa^T)
  k,v = (x @ (Wkvb diag(w) Wkva_c)^T) * inv_rms(x Wkva_c^T)
Each core only computes the absorbed GEMMs for its 2 heads instead of the
replicated LoRA-A GEMMs. The per-token inv_rms scalars still need the full
latent rows; those are computed seq-sharded (256 tokens per core) and
exchanged with a single 16 KB AllGather whose latency hides under the
absorbed GEMMs. inv_rms scaling is deferred until after the gather (rope
commutes with per-token scaling).

Precision plan (tolerance 2e-2, validated offline in fp8_sim.py):
- stats GEMMs: fp8e4 DoubleRow, 1-term (inv_rms only needs ~0.2%)
- absorbed GEMMs: fp8e4 DoubleRow, 3-term compensated
  (x8@w_hi + r8@w_hi + x8@w_lo), host-prepared splits
- attention + o_proj: bf16 operands, fp32 PSUM
Weights are pre-scaled by 2^5 to center e4m3; the factor is folded into the
Sqrt activation scale (stats), the Exp scale (q·k carries 2^10), and into
w_o on the host (v path). Softmax denominators accumulate on DVE. o_proj
partials are summed on the host across cores (bf16 partial writes).
"""
import numpy as np

import concourse.bass as bass
import concourse.tile as tile
from concourse import bacc, mybir
from concourse.bass_utils import run_bass_kernel_spmd

F32 = mybir.dt.float32
F32R = mybir.dt.float32r
BF16 = mybir.dt.bfloat16
F8 = mybir.dt.float8e4

HIDDEN = 2048
S = 2048
NUM_HEADS = 16
Q_LORA = 1536
KV_LORA = 512
NOPE = 128
ROPE = 64
VD = 128
QD = NOPE + ROPE            # 192
SCALE = QD ** -0.5
EPS = 1e-6
ROPE_THETA = 10000.0

NCORES = 8
HPC = NUM_HEADS // NCORES   # 2
SB = 512
NSB = S // SB               # 4
KT = HIDDEN // 128          # 16
NJ = KT // 2                # 8 DoubleRow k-pairs
SEQB = S // NCORES          # 256 stats tokens per core
QMT = (HPC * QD) // 128     # 3
KMT = (HPC * NOPE) // 128   # 2
WS = 32.0                   # 2^5 weight pre-scale for e4m3

_CACHE = {}
LAST_RESULT = None


def _build_program():
    nc = bacc.Bacc("TRN2", target_bir_lowering=False, debug=False,
                   num_devices=NCORES)
    dt = nc.dram_tensor
    d = {
        "xs": dt("xs", [128, KT, SEQB], F8, kind="ExternalInput").ap(),
        "wqa": dt("wqa_t", [128, KT, Q_LORA], F8, kind="ExternalInput").ap(),
        "wkvac": dt("wkvac_t", [128, KT, KV_LORA], F8, kind="ExternalInput").ap(),
        "xh": dt("xh", [128, KT, S], F8, kind="ExternalInput").ap(),
        "xl": dt("xl", [128, KT, S], F8, kind="ExternalInput").ap(),
        "bth": dt("bth", [128, KT, HPC * QD], F8, kind="ExternalInput").ap(),
        "btl": dt("btl", [128, KT, HPC * QD], F8, kind="ExternalInput").ap(),
        "ckth": dt("ckth", [128, KT, HPC * NOPE], F8, kind="ExternalInput").ap(),
        "cktl": dt("cktl", [128, KT, HPC * NOPE], F8, kind="ExternalInput").ap(),
        "cvth": dt("cvth", [128, KT, HPC * VD], F8, kind="ExternalInput").ap(),
        "cvtl": dt("cvtl", [128, KT, HPC * VD], F8, kind="ExternalInput").ap(),
        "wpeh": dt("wpeh", [128, KT, ROPE], F8, kind="ExternalInput").ap(),
        "wpel": dt("wpel", [128, KT, ROPE], F8, kind="ExternalInput").ap(),
        "wo": dt("wo_t", [128, HPC, HIDDEN], BF16, kind="ExternalInput").ap(),
        "cos": dt("cosd", [128, S], BF16, kind="ExternalInput").ap(),
        "sin": dt("sind", [128, S], BF16, kind="ExternalInput").ap(),
        "msk": dt("mask", [128, 4, SB], BF16, kind="ExternalInput").ap(),
        "onesb": dt("onesb", [128, 128], BF16, kind="ExternalInput").ap(),
        "onesr": dt("onesr", [128, 128], F32R, kind="ExternalInput").ap(),
        "rotq": dt("rotq", [128, 128], BF16, kind="ExternalInput").ap(),
        "dupx": dt("dupx", [64, 128], BF16, kind="ExternalInput").ap(),
        "duprot": dt("duprot", [64, 128], BF16, kind="ExternalInput").ap(),
        "out": dt("out", [S, HIDDEN], BF16, kind="ExternalOutput").ap(),
    }
    with tile.TileContext(nc) as tc:
        _mla(tc, d)
    nc.compile()
    return nc


def _rearr(ap):
    return ap.rearrange("(t p) f -> p t f", p=128)


def _mla(tc, d):
    nc = tc.nc
    Exp = mybir.ActivationFunctionType.Exp
    Sqrt = mybir.ActivationFunctionType.Sqrt
    DR = mybir.MatmulPerfMode.DoubleRow
    QLT = Q_LORA // 128     # 12
    CT = KV_LORA // 128     # 4

    with nc.allow_low_precision(reason="fp8/bf16 matmul pipeline with fp32 "
                                "accumulation; tolerance is 2e-2"), \
         tc.tile_pool(name="pxb", bufs=2) as pxb, \
         tc.tile_pool(name="pconst", bufs=1) as pc, \
         tc.tile_pool(name="pqkv", bufs=1) as pqkv, \
         tc.tile_pool(name="pdram", bufs=1, space="DRAM") as pdram:
        # ---- DMAs in PE-consumption order: block-0 inputs lead ----
        xh = {}
        xl = {}
        xh[0] = pxb.tile([128, KT, SB], F8, tag="xh", name="xh0")
        bth = pc.tile([128, KT, HPC * QD], F8)
        xl[0] = pxb.tile([128, KT, SB], F8, tag="xl", name="xl0")
        btl = pc.tile([128, KT, HPC * QD], F8)
        for kk in (slice(0, 8), slice(8, KT)):
            nc.sync.dma_start(out=xh[0][:, kk, :], in_=d["xh"][:, kk, 0:SB])
            nc.sync.dma_start(out=bth[:, kk, :], in_=d["bth"][:, kk, :])
            nc.sync.dma_start(out=xl[0][:, kk, :], in_=d["xl"][:, kk, 0:SB])
            nc.sync.dma_start(out=btl[:, kk, :], in_=d["btl"][:, kk, :])
        ones_b = pc.tile([128, 128], BF16)
        nc.sync.dma_start(out=ones_b, in_=d["onesb"])
        ckth = pc.tile([128, KT, HPC * NOPE], F8)
        nc.sync.dma_start(out=ckth, in_=d["ckth"])
        cktl = pc.tile([128, KT, HPC * NOPE], F8)
        nc.sync.dma_start(out=cktl, in_=d["cktl"])
        wpeh = pc.tile([128, KT, ROPE], F8)
        nc.sync.dma_start(out=wpeh, in_=d["wpeh"])
        wpel = pc.tile([128, KT, ROPE], F8)
        nc.sync.dma_start(out=wpel, in_=d["wpel"])
        cvth = pc.tile([128, KT, HPC * VD], F8)
        nc.sync.dma_start(out=cvth, in_=d["cvth"])
        cvtl = pc.tile([128, KT, HPC * VD], F8)
        nc.sync.dma_start(out=cvtl, in_=d["cvtl"])
        rotq = pc.tile([128, 128], BF16)
        nc.sync.dma_start(out=rotq, in_=d["rotq"])
        dupx = pc.tile([64, 128], BF16)
        nc.sync.dma_start(out=dupx, in_=d["dupx"])
        duprot = pc.tile([64, 128], BF16)
        nc.sync.dma_start(out=duprot, in_=d["duprot"])
        cosd = pc.tile([128, S], BF16)
        nc.sync.dma_start(out=cosd, in_=d["cos"])
        sind = pc.tile([128, S], BF16)
        nc.sync.dma_start(out=sind, in_=d["sin"])
        eps1 = pc.tile([1, 1], F32)
        nc.vector.memset(eps1, EPS)

        # ---- persistent per-head tensors ----
        qn = [pqkv.tile([128, S], BF16, tag=f"qn{h}", name=f"qn{h}")
              for h in range(HPC)]
        kn = [pqkv.tile([128, S], BF16, tag=f"kn{h}", name=f"kn{h}")
              for h in range(HPC)]
        qpe = pqkv.tile([128, S], BF16, tag="qpe")
        kpd = pqkv.tile([128, S], BF16, tag="kpd")
        vst = pqkv.tile([128, S // 128, HPC * VD], BF16, tag="vst")
        ao = [pqkv.tile([128, S], BF16, tag=f"ao{h}", name=f"ao{h}")
              for h in range(HPC)]
        invqb = pqkv.tile([128, S], BF16, tag="invqb")
        invcb = pqkv.tile([128, S], BF16, tag="invcb")
        invq_row = pqkv.tile([1, NCORES, SEQB], F32R, tag="invq_row")
        invc_row = pqkv.tile([1, NCORES, SEQB], F32R, tag="invc_row")
        invc_col = pqkv.tile([128, NCORES, SEQB // 128], F32, tag="invc_col")

        cc_in = pdram.tile([2, SEQB], F32R)
        cc_out = pdram.tile([NCORES, 2, SEQB], F32R)

        with tc.tile_pool(name="p2sb", bufs=2) as p2sb, \
             tc.tile_pool(name="pp2", bufs=3, space="PSUM") as pp2, \
             tc.tile_pool(name="pp2v", bufs=2, space="PSUM") as pp2v:

            def absorbed(p, wh, wl, ws, xhb, xlb, nt=3):
                """nt-term compensated fp8 DoubleRow accumulation into psum.
                3-term: x8@w_hi + r8@w_hi + x8@w_lo; 2-term drops w_lo."""
                terms = [(xhb, wh), (xlb, wh), (xhb, wl)][:nt]
                for t, (xx, ww) in enumerate(terms):
                    for j in range(NJ):
                        nc.tensor.matmul(
                            p, ww[:, 2 * j:2 * j + 2, ws],
                            xx[:, 2 * j:2 * j + 2, :],
                            start=(t == 0 and j == 0),
                            stop=(t == nt - 1 and j == NJ - 1), perf_mode=DR)

            def block_proj(b):
                cols = bass.ts(b, SB)
                if b >= 3:
                    xh[b] = pxb.tile([128, KT, SB], F8, tag="xh", name=f"xh{b}")
                    nc.sync.dma_start(out=xh[b], in_=d["xh"][:, :, cols])
                    xl[b] = pxb.tile([128, KT, SB], F8, tag="xl", name=f"xl{b}")
                    nc.sync.dma_start(out=xl[b], in_=d["xl"][:, :, cols])

                qdst = [qn[0], qn[1]]
                for mt in range(QMT):
                    p_q = pp2.tile([128, SB], F32, tag="mm")
                    absorbed(p_q[:], bth, btl,
                             slice(mt * 128, (mt + 1) * 128), xh[b], xl[b])
                    if mt < 2:
                        nc.any.tensor_copy(qdst[mt][:, cols], p_q[:])
                    else:
                        qpe_u = p2sb.tile([128, SB], BF16, tag="t0")
                        nc.any.tensor_copy(qpe_u[:], p_q[:])
                        p_rq = pp2.tile([128, SB], F32, tag="mm")
                        nc.tensor.matmul(p_rq[:], rotq[:], qpe_u[:],
                                         start=True, stop=True)
                        t1 = p2sb.tile([128, SB], BF16, tag="t1")
                        nc.vector.tensor_mul(t1[:], qpe_u[:], cosd[:, cols])
                        t2 = p2sb.tile([128, SB], BF16, tag="t2")
                        nc.vector.tensor_mul(t2[:], p_rq[:], sind[:, cols])
                        nc.vector.tensor_add(qpe[:, cols], t1[:], t2[:])

                for mt in range(KMT):
                    p_k = pp2.tile([128, SB], F32, tag="mm")
                    absorbed(p_k[:], ckth, cktl,
                             slice(mt * 128, (mt + 1) * 128), xh[b], xl[b],
                             nt=2)
                    nc.any.tensor_copy(kn[mt][:, cols], p_k[:])

                p_pe = pp2.tile([128, SB], F32, tag="mm")
                absorbed(p_pe[:ROPE, :], wpeh, wpel, slice(0, ROPE),
                         xh[b], xl[b], nt=2)
                kpe_u = p2sb.tile([ROPE, SB], BF16, tag="t3")
                nc.any.tensor_copy(kpe_u[:], p_pe[:ROPE, :])
                p_x = pp2.tile([128, SB], F32, tag="mm")
                nc.tensor.matmul(p_x[:], dupx[:], kpe_u[:], start=True, stop=True)
                xb16 = p2sb.tile([128, SB], BF16, tag="t4")
                nc.scalar.activation(xb16[:], p_x[:],
                                     mybir.ActivationFunctionType.Copy)
                p_r = pp2.tile([128, SB], F32, tag="mm")
                nc.tensor.matmul(p_r[:], duprot[:], kpe_u[:], start=True, stop=True)
                rb16 = p2sb.tile([128, SB], BF16, tag="t5")
                nc.scalar.activation(rb16[:], p_r[:],
                                     mybir.ActivationFunctionType.Copy)
                t1 = p2sb.tile([128, SB], BF16, tag="t1")
                nc.vector.tensor_mul(t1[:], xb16[:], cosd[:, cols])
                t2 = p2sb.tile([128, SB], BF16, tag="t2")
                nc.vector.tensor_mul(t2[:], rb16[:], sind[:, cols])
                nc.vector.tensor_add(kpd[:, cols], t1[:], t2[:])

                for t4 in range(SB // 128):
                    sl = slice(t4 * 128, (t4 + 1) * 128)
                    p_v = pp2v.tile([128, HPC * VD], F32, tag="vv")
                    vterms = [(xh[b], cvth), (xl[b], cvth), (xh[b], cvtl)]
                    for t, (xx, ww) in enumerate(vterms):
                        for j in range(NJ):
                            nc.tensor.matmul(
                                p_v[:], xx[:, 2 * j:2 * j + 2, sl],
                                ww[:, 2 * j:2 * j + 2, :],
                                start=(t == 0 and j == 0),
                                stop=(t == 2 and j == NJ - 1), perf_mode=DR)
                    nc.any.tensor_copy(vst[:, b * (SB // 128) + t4, :], p_v[:])

            # block 0 first: its inputs lead the DMA queue
            block_proj(0)

            # ---- phase 1 (stats) in the middle; collective hides ----
            with tc.tile_pool(name="pstats", bufs=1) as pst, \
                 tc.tile_pool(name="p1sb", bufs=3) as p1sb, \
                 tc.tile_pool(name="p1st", bufs=1) as p1st, \
                 tc.tile_pool(name="pp1", bufs=2, space="PSUM") as pp1, \
                 tc.tile_pool(name="pp1s", bufs=1, space="PSUM") as pp1s:
                xs = pst.tile([128, KT, SEQB], F8)
                nc.sync.dma_start(out=xs, in_=d["xs"])
                wqa = pst.tile([128, KT, Q_LORA], F8)
                for ch in range(3):
                    cs = slice(ch * 512, (ch + 1) * 512)
                    nc.sync.dma_start(out=wqa[:, :, cs],
                                      in_=d["wqa"][:, :, cs])
                wkvac = pst.tile([128, KT, KV_LORA], F8)
                nc.sync.dma_start(out=wkvac, in_=d["wkvac"])

                p_qs = pp1s.tile([1, SEQB], F32, tag="stat", name="p_qs")
                for m in range(QLT):
                    p_a = pp1.tile([128, SEQB], F32, tag="acc")
                    for j in range(NJ):
                        nc.tensor.matmul(p_a[:], wqa[:, 2 * j:2 * j + 2,
                                                     m * 128:(m + 1) * 128],
                                         xs[:, 2 * j:2 * j + 2, :],
                                         start=(j == 0), stop=(j == NJ - 1),
                                         perf_mode=DR)
                    ql = p1sb.tile([128, SEQB], BF16, tag="ql")
                    nc.any.tensor_copy(ql[:], p_a[:])
                    sq = p1sb.tile([128, SEQB], BF16, tag="sq")
                    nc.vector.tensor_mul(sq[:], ql[:], ql[:])
                    nc.tensor.matmul(p_qs[:], ones_b[:, 0:1], sq[:],
                                     start=(m == 0), stop=(m == QLT - 1))
                qs_s = p1st.tile([1, SEQB], F32, tag="s1")
                nc.scalar.activation(qs_s[:], p_qs[:], Sqrt,
                                     scale=1.0 / (Q_LORA * WS * WS), bias=eps1[:])
                invq_s = p1st.tile([1, SEQB], F32R, tag="s2")
                nc.vector.reciprocal(invq_s[:], qs_s[:])
                nc.gpsimd.dma_start(out=cc_in[0:1, :], in_=invq_s[:])

                p_cs = pp1s.tile([1, SEQB], F32, tag="stat", name="p_cs")
                for m in range(CT):
                    p_a = pp1.tile([128, SEQB], F32, tag="acc")
                    for j in range(NJ):
                        nc.tensor.matmul(p_a[:], wkvac[:, 2 * j:2 * j + 2,
                                                       m * 128:(m + 1) * 128],
                                         xs[:, 2 * j:2 * j + 2, :],
                                         start=(j == 0), stop=(j == NJ - 1),
                                         perf_mode=DR)
                    cl = p1sb.tile([128, SEQB], BF16, tag="ql")
                    nc.any.tensor_copy(cl[:], p_a[:])
                    sq = p1sb.tile([128, SEQB], BF16, tag="sq")
                    nc.vector.tensor_mul(sq[:], cl[:], cl[:])
                    nc.tensor.matmul(p_cs[:], ones_b[:, 0:1], sq[:],
                                     start=(m == 0), stop=(m == CT - 1))
                cs_s = p1st.tile([1, SEQB], F32, tag="s3")
                nc.scalar.activation(cs_s[:], p_cs[:], Sqrt,
                                     scale=1.0 / (KV_LORA * WS * WS), bias=eps1[:])
                invc_s = p1st.tile([1, SEQB], F32R, tag="s4")
                nc.vector.reciprocal(invc_s[:], cs_s[:])
                nc.gpsimd.dma_start(out=cc_in[1:2, :], in_=invc_s[:])

                nc.gpsimd.collective_compute(
                    "AllGather", mybir.AluOpType.bypass,
                    replica_groups=[list(range(NCORES))],
                    ins=[cc_in[:].opt()], outs=[cc_out[:].opt()])
                nc.sync.dma_start(out=invq_row, in_=cc_out[:, 0, :])
                nc.sync.dma_start(out=invc_row, in_=cc_out[:, 1, :])
                for g in range(NCORES):
                    nc.gpsimd.dma_start(
                        out=invc_col[:, g, :],
                        in_=cc_out[g, 1, :].rearrange("(t p) -> p t", p=128))

            # prefetch blocks 1-2 x tiles, then late-need constants
            for bb in (1, 2):
                cols = bass.ts(bb, SB)
                xh[bb] = pxb.tile([128, KT, SB], F8, tag="xh", name=f"xh{bb}")
                nc.sync.dma_start(out=xh[bb], in_=d["xh"][:, :, cols])
                xl[bb] = pxb.tile([128, KT, SB], F8, tag="xl", name=f"xl{bb}")
                nc.sync.dma_start(out=xl[bb], in_=d["xl"][:, :, cols])
            ones_r = pc.tile([128, 128], F32R)
            nc.sync.dma_start(out=ones_r, in_=d["onesr"])
            msk = pc.tile([128, 4, SB], BF16)
            nc.sync.dma_start(out=msk, in_=d["msk"])
            wo = pc.tile([128, HPC, HIDDEN], BF16)
            nc.sync.dma_start(out=wo, in_=d["wo"])

            def scale_block(b):
                # inv_rms application for block b (needs the AllGather)
                cols = bass.ts(b, SB)
                p_bq = pp2.tile([128, SB], F32, tag="mm")
                nc.tensor.matmul(p_bq[:], ones_r[0:1, :],
                                 invq_row[0:1, 2 * b:2 * b + 2, :],
                                 start=True, stop=True)
                nc.any.tensor_copy(invqb[:, cols], p_bq[:])
                p_bc = pp2.tile([128, SB], F32, tag="mm")
                nc.tensor.matmul(p_bc[:], ones_r[0:1, :],
                                 invc_row[0:1, 2 * b:2 * b + 2, :],
                                 start=True, stop=True)
                nc.any.tensor_copy(invcb[:, cols], p_bc[:])
                nc.vector.tensor_mul(qn[0][:, cols], qn[0][:, cols], invqb[:, cols])
                nc.vector.tensor_mul(qn[1][:, cols], qn[1][:, cols], invqb[:, cols])
                nc.vector.tensor_mul(qpe[:, cols], qpe[:, cols], invqb[:, cols])
                nc.vector.tensor_mul(kn[0][:, cols], kn[0][:, cols], invcb[:, cols])
                nc.vector.tensor_mul(kn[1][:, cols], kn[1][:, cols], invcb[:, cols])
                for st in range(4 * b, 4 * (b + 1)):
                    nc.vector.tensor_scalar_mul(
                        vst[:, st, :], vst[:, st, :],
                        invc_col[:, st // 2, st % 2:st % 2 + 1])

            block_proj(1)
            block_proj(2)
            scale_block(0)
            scale_block(1)
            block_proj(3)
            scale_block(2)
            scale_block(3)

        # ------- phase 3: attention per (block, head) + fused o_proj -------
        # q and k each carry the 2^5 weight pre-scale; fold 2^-10 into Exp.
        ESCALE = SCALE / (WS * WS)
        with tc.tile_pool(name="pexp", bufs=3) as pexp, \
             tc.tile_pool(name="pes", bufs=2) as pes, \
             tc.tile_pool(name="pbn", bufs=2) as pbn, \
             tc.tile_pool(name="pout", bufs=4) as pout, \
             tc.tile_pool(name="ppS", bufs=3, space="PSUM") as ppS, \
             tc.tile_pool(name="ppO", bufs=2, space="PSUM") as ppO, \
             tc.tile_pool(name="ppB", bufs=1, space="PSUM") as ppB, \
             tc.tile_pool(name="ppC", bufs=2, space="PSUM") as ppC:
            for qb in range(NSB):
                qcols = bass.ts(qb, SB)
                nk = 4 * (qb + 1)
                for h in range(HPC):
                    hp = slice(64 * h, 64 * h + 64)
                    p_o = ppO.tile([128, SB], F32, tag="o")
                    esum = [pes.tile([128, SB], F32R, tag=f"es{par}",
                                     name=f"es{par}") for par in range(2)]
                    for ik in range(nk):
                        kc = slice(ik * 128, (ik + 1) * 128)
                        r = ik - 4 * qb
                        # diagonal tiles: only queries >= 128r can attend this
                        # k-tile; trim the matmul/exp width (qb>0 so the
                        # first two chain tiles stay full width)
                        lo = 128 * r if (r >= 1 and qb > 0) else 0
                        qsub = slice(qb * SB + lo, (qb + 1) * SB)
                        sub = slice(lo, SB)
                        p_s = ppS.tile([128, SB], F32, tag="s")
                        nc.tensor.matmul(p_s[:, sub], kn[h][:, kc],
                                         qn[h][:, qsub], start=True, stop=False)
                        nc.tensor.matmul(p_s[:, sub], kpd[hp, kc],
                                         qpe[hp, qsub], start=False, stop=True)
                        e = pexp.tile([128, SB], BF16, tag="e")
                        nc.scalar.activation(e[:, sub], p_s[:, sub], Exp,
                                             scale=ESCALE)
                        if r >= 0:
                            nc.vector.tensor_mul(e[:, sub], e[:, sub],
                                                 msk[:, r, sub])
                        nc.tensor.matmul(p_o[:, sub],
                                         vst[:, ik, h * VD:(h + 1) * VD],
                                         e[:, sub], start=(ik == 0),
                                         stop=(ik == nk - 1))
                        es = esum[ik % 2]
                        if ik < 2:
                            nc.vector.tensor_copy(es[:], e[:])
                        else:
                            nc.vector.tensor_add(es[:, sub], es[:, sub],
                                                 e[:, sub])
                    p_bc = ppB.tile([128, SB], F32, tag="bc")
                    p_d = p_bc[0:1, :]
                    nc.tensor.matmul(p_d, ones_r[:, 0:1], esum[0][:],
                                     start=True, stop=False)
                    nc.tensor.matmul(p_d, ones_r[:, 0:1], esum[1][:],
                                     start=False, stop=True)
                    rec_s = pbn.tile([1, SB], F32R, tag="rec")
                    nc.vector.reciprocal(rec_s[:], p_d)
                    nc.tensor.matmul(p_bc[:], ones_r[0:1, :], rec_s[:],
                                     start=True, stop=True)
                    recb = pbn.tile([128, SB], BF16, tag="recb")
                    nc.any.tensor_copy(recb[:], p_bc[:])
                    nc.vector.tensor_mul(ao[h][:, qcols], p_o[:], recb[:])
                for st in range(qb * (SB // 128), (qb + 1) * (SB // 128)):
                    sc = slice(st * 128, (st + 1) * 128)
                    ot = pout.tile([128, HIDDEN], BF16, tag="ot")
                    fine = (qb == NSB - 1 and st == (qb + 1) * (SB // 128) - 1)
                    for nb in range(HIDDEN // SB):
                        ncols = bass.ts(nb, SB)
                        p_c = ppC.tile([128, SB], F32, tag="c")
                        for h in range(HPC):
                            nc.tensor.matmul(p_c[:], ao[h][:, sc], wo[:, h, ncols],
                                             start=(h == 0), stop=(h == HPC - 1))
                        nc.any.tensor_copy(ot[:, ncols], p_c[:])
                        if fine:
                            # pipeline the final drain with per-chunk DMAs
                            nc.sync.dma_start(out=d["out"][sc, ncols],
                                              in_=ot[:, ncols])
                    if not fine:
                        nc.sync.dma_start(out=d["out"][sc, :], in_=ot[:])


def _host_constants():
    inv_freq = 1.0 / (ROPE_THETA ** (np.arange(0, ROPE, dtype=np.float32)[0::2] / ROPE))
    t = np.arange(S, dtype=np.float32)
    freqs = np.outer(t, inv_freq)
    emb = np.concatenate([freqs, freqs], -1)          # [S, 64]
    cos, sin = np.cos(emb), np.sin(emb)
    cosd = np.concatenate([cos.T, cos.T], 0).astype(np.float32)   # [128, S]
    sind = np.concatenate([sin.T, sin.T], 0).astype(np.float32)

    msk = np.zeros((128, 4, SB), np.float32)
    for r in range(4):
        for p in range(128):
            k_idx = p + 128 * r
            if k_idx < SB:
                msk[p, r, k_idx:] = 1.0               # keep where k <= q

    Q = np.zeros((64, 64), np.float32)
    for i in range(32):
        Q[i, i + 32] = -1.0
        Q[i + 32, i] = 1.0
    P = np.zeros((128, 128), np.float32)
    P[:64, :64] = Q
    P[64:, 64:] = Q
    rotq = P.T.copy()
    D = np.concatenate([np.eye(64, dtype=np.float32)] * 2, 0)     # [128, 64]
    dupx = D.T.copy()                                  # [64, 128]
    duprot = np.concatenate([Q, Q], 0).T.copy()        # [64, 128]
    return cosd, sind, msk, rotq, dupx, duprot


def kernel(hidden_states, w_q_a, q_a_weight, w_q_b, w_kv_a, kv_a_weight,
           w_kv_b, w_o):
    global LAST_RESULT
    import ml_dtypes
    bf16 = ml_dtypes.bfloat16
    f8 = ml_dtypes.float8_e4m3
    if "nc" not in _CACHE:
        _CACHE["nc"] = _build_program()
    nc = _CACHE["nc"]

    def b(a):
        return np.ascontiguousarray(np.asarray(a, np.float32).astype(bf16))

    def e8(a):
        return np.ascontiguousarray(np.asarray(a, np.float32).astype(f8))

    def pre(a):
        """[HIDDEN, F] -> [128, KT, F] partition-major prearrangement."""
        a = np.asarray(a)
        t = a.shape[0] // 128
        return np.ascontiguousarray(
            a.reshape(t, 128, a.shape[1]).transpose(1, 0, 2))

    def split8(a):
        """hi/lo fp8 split of an already-scaled array, prearranged."""
        a = np.asarray(a, np.float32)
        hi = a.astype(f8)
        lo = (a - hi.astype(np.float32)).astype(f8)
        return pre(hi), pre(lo)

    x = np.asarray(hidden_states, np.float32)[0]       # [S, 2048]
    xt = x.T                                           # [2048, S]
    wqa = np.asarray(w_q_a, np.float32)                # [1536, 2048]
    wkva = np.asarray(w_kv_a, np.float32)              # [576, 2048]
    wqb_eff = np.asarray(w_q_b, np.float32) * np.asarray(q_a_weight, np.float32)[None, :]
    wkvb_eff = np.asarray(w_kv_b, np.float32) * np.asarray(kv_a_weight, np.float32)[None, :]
    won = np.asarray(w_o, np.float32)

    cosd, sind, msk, rotq, dupx, duprot = _host_constants()
    onesm = np.ones((128, 128), np.float32)
    xh8, xl8 = split8(xt)
    wpeh8, wpel8 = split8(WS * wkva[KV_LORA:].T)
    shared = {"onesb": b(onesm), "onesr": onesm,
              "xh": xh8, "xl": xl8,
              "wqa_t": pre(e8(WS * wqa.T)), "wkvac_t": pre(e8(WS * wkva[:KV_LORA].T)),
              "wpeh": wpeh8, "wpel": wpel8,
              "cosd": b(cosd), "sind": b(sind), "mask": b(msk),
              "rotq": b(rotq), "dupx": b(dupx), "duprot": b(duprot)}

    in_maps = []
    for c in range(NCORES):
        h0, h1 = HPC * c, HPC * c + 1
        rows_q = np.concatenate(
            [wqb_eff[h0 * QD:h0 * QD + NOPE],
             wqb_eff[h1 * QD:h1 * QD + NOPE],
             wqb_eff[h0 * QD + NOPE:(h0 + 1) * QD],
             wqb_eff[h1 * QD + NOPE:(h1 + 1) * QD]], 0)          # [384, 1536]
        B = rows_q @ wqa                                          # [384, 2048]
        rows_k = np.concatenate(
            [wkvb_eff[h * (NOPE + VD):h * (NOPE + VD) + NOPE] for h in (h0, h1)], 0)
        Ck = rows_k @ wkva[:KV_LORA]
        rows_v = np.concatenate(
            [wkvb_eff[h * (NOPE + VD) + NOPE:(h + 1) * (NOPE + VD)]
             for h in (h0, h1)], 0)
        Cv = rows_v @ wkva[:KV_LORA]
        # v path carries the 2^5 Cv pre-scale: fold 2^-5 into w_o
        wo_t = np.concatenate(
            [won[:, h * VD:(h + 1) * VD] for h in (h0, h1)], 1).T / WS
        bth8, btl8 = split8(WS * B.T)
        ckth8, cktl8 = split8(WS * Ck.T)
        cvth8, cvtl8 = split8(WS * Cv.T)
        im = dict(shared)
        im.update({"bth": bth8, "btl": btl8, "ckth": ckth8, "cktl": cktl8,
                   "cvth": cvth8, "cvtl": cvtl8,
                   "wo_t": pre(b(wo_t)).reshape(128, HPC, HIDDEN),
                   "xs": pre(e8(xt[:, SEQB * c:SEQB * (c + 1)]))})
        in_maps.append(im)

    res = run_bass_kernel_spmd(nc, in_maps, list(range(NCORES)))
    LAST_RESULT = res
    out = np.zeros((S, HIDDEN), np.float32)
    for c in range(NCORES):
        out += res.results[c]["out"].astype(np.float32)
    return out.reshape(1, S, HIDDEN)


# revision 5
# speedup vs baseline: 1.0288x; 1.0078x over previous
"""MLA forward on 8 trn2 cores — absorbed-weight tensor-parallel version.

Key algebraic move: RMSNorm(z) = z * inv_rms(z) * w with inv_rms a per-token
scalar, so the LoRA up-projections absorb the down-projections on the host:
  q   = (x @ (Wqb diag(w) Wqa)^T) * inv_rms(x Wqa^T)
  k,v = (x @ (Wkvb diag(w) Wkva_c)^T) * inv_rms(x Wkva_c^T)
Each core only computes the absorbed GEMMs for its 2 heads instead of the
replicated LoRA-A GEMMs. The per-token inv_rms scalars still need the full
latent rows; those are computed seq-sharded (256 tokens per core) and
exchanged with a single 16 KB AllGather whose latency hides under the
absorbed GEMMs. inv_rms scaling is deferred until after the gather (rope
commutes with per-token scaling).

Precision plan (tolerance 2e-2, validated offline in fp8_sim.py):
- stats GEMMs: fp8e4 DoubleRow, 1-term (inv_rms only needs ~0.2%)
- absorbed GEMMs: fp8e4 DoubleRow, 3-term compensated
  (x8@w_hi + r8@w_hi + x8@w_lo), host-prepared splits
- attention + o_proj: bf16 operands, fp32 PSUM
Weights are pre-scaled by 2^5 to center e4m3; the factor is folded into the
Sqrt activation scale (stats), the Exp scale (q·k carries 2^10), and into
w_o on the host (v path). Softmax denominators accumulate on DVE. o_proj
partials are summed on the host across cores (bf16 partial writes).
"""
import numpy as np

import concourse.bass as bass
import concourse.tile as tile
from concourse import bacc, mybir
from concourse.bass_utils import run_bass_kernel_spmd

F32 = mybir.dt.float32
F32R = mybir.dt.float32r
BF16 = mybir.dt.bfloat16
F8 = mybir.dt.float8e4

HIDDEN = 2048
S = 2048
NUM_HEADS = 16
Q_LORA = 1536
KV_LORA = 512
NOPE = 128
ROPE = 64
VD = 128
QD = NOPE + ROPE            # 192
SCALE = QD ** -0.5
EPS = 1e-6
ROPE_THETA = 10000.0

NCORES = 8
HPC = NUM_HEADS // NCORES   # 2
SB = 512
NSB = S // SB               # 4
KT = HIDDEN // 128          # 16
NJ = KT // 2                # 8 DoubleRow k-pairs
SEQB = S // NCORES          # 256 stats tokens per core
QMT = (HPC * QD) // 128     # 3
KMT = (HPC * NOPE) // 128   # 2
WS = 32.0                   # 2^5 weight pre-scale for e4m3

_CACHE = {}
LAST_RESULT = None


def _build_program():
    nc = bacc.Bacc("TRN2", target_bir_lowering=False, debug=False,
                   num_devices=NCORES)
    dt = nc.dram_tensor
    d = {
        "xs": dt("xs", [128, KT, SEQB], F8, kind="ExternalInput").ap(),
        "wqa": dt("wqa_t", [128, KT, Q_LORA], F8, kind="ExternalInput").ap(),
        "wkvac": dt("wkvac_t", [128, KT, KV_LORA], F8, kind="ExternalInput").ap(),
        "xh": dt("xh", [128, KT, S], F8, kind="ExternalInput").ap(),
        "xl": dt("xl", [128, KT, S], F8, kind="ExternalInput").ap(),
        "bth": dt("bth", [128, KT, HPC * QD], F8, kind="ExternalInput").ap(),
        "btl": dt("btl", [128, KT, HPC * QD], F8, kind="ExternalInput").ap(),
        "ckth": dt("ckth", [128, KT, HPC * NOPE], F8, kind="ExternalInput").ap(),
        "cktl": dt("cktl", [128, KT, HPC * NOPE], F8, kind="ExternalInput").ap(),
        "cvth": dt("cvth", [128, KT, HPC * VD], F8, kind="ExternalInput").ap(),
        "cvtl": dt("cvtl", [128, KT, HPC * VD], F8, kind="ExternalInput").ap(),
        "wpeh": dt("wpeh", [128, KT, ROPE], F8, kind="ExternalInput").ap(),
        "wpel": dt("wpel", [128, KT, ROPE], F8, kind="ExternalInput").ap(),
        "wo": dt("wo_t", [128, HPC, HIDDEN], BF16, kind="ExternalInput").ap(),
        "cos": dt("cosd", [128, S], BF16, kind="ExternalInput").ap(),
        "sin": dt("sind", [128, S], BF16, kind="ExternalInput").ap(),
        "msk": dt("mask", [128, 4, SB], BF16, kind="ExternalInput").ap(),
        "onesb": dt("onesb", [128, 128], BF16, kind="ExternalInput").ap(),
        "onesr": dt("onesr", [128, 128], F32R, kind="ExternalInput").ap(),
        "rotq": dt("rotq", [128, 128], BF16, kind="ExternalInput").ap(),
        "dupx": dt("dupx", [64, 128], BF16, kind="ExternalInput").ap(),
        "duprot": dt("duprot", [64, 128], BF16, kind="ExternalInput").ap(),
        "out": dt("out", [S, HIDDEN], BF16, kind="ExternalOutput").ap(),
    }
    with tile.TileContext(nc) as tc:
        _mla(tc, d)
    nc.compile()
    return nc


def _rearr(ap):
    return ap.rearrange("(t p) f -> p t f", p=128)


def _mla(tc, d):
    nc = tc.nc
    Exp = mybir.ActivationFunctionType.Exp
    Sqrt = mybir.ActivationFunctionType.Sqrt
    DR = mybir.MatmulPerfMode.DoubleRow
    QLT = Q_LORA // 128     # 12
    CT = KV_LORA // 128     # 4

    with nc.allow_low_precision(reason="fp8/bf16 matmul pipeline with fp32 "
                                "accumulation; tolerance is 2e-2"), \
         tc.tile_pool(name="pxb", bufs=2) as pxb, \
         tc.tile_pool(name="pconst", bufs=1) as pc, \
         tc.tile_pool(name="pqkv", bufs=1) as pqkv, \
         tc.tile_pool(name="pdram", bufs=1, space="DRAM") as pdram:
        # ---- DMAs in PE-consumption order: block-0 inputs lead ----
        xh = {}
        xl = {}
        xh[0] = pxb.tile([128, KT, SB], F8, tag="xh", name="xh0")
        bth = pc.tile([128, KT, HPC * QD], F8)
        xl[0] = pxb.tile([128, KT, SB], F8, tag="xl", name="xl0")
        btl = pc.tile([128, KT, HPC * QD], F8)
        for kk in (slice(0, 8), slice(8, KT)):
            nc.sync.dma_start(out=xh[0][:, kk, :], in_=d["xh"][:, kk, 0:SB])
            nc.sync.dma_start(out=bth[:, kk, :], in_=d["bth"][:, kk, :])
            nc.sync.dma_start(out=xl[0][:, kk, :], in_=d["xl"][:, kk, 0:SB])
            nc.sync.dma_start(out=btl[:, kk, :], in_=d["btl"][:, kk, :])
        ones_b = pc.tile([128, 128], BF16)
        nc.sync.dma_start(out=ones_b, in_=d["onesb"])
        ckth = pc.tile([128, KT, HPC * NOPE], F8)
        nc.sync.dma_start(out=ckth, in_=d["ckth"])
        cktl = pc.tile([128, KT, HPC * NOPE], F8)
        nc.sync.dma_start(out=cktl, in_=d["cktl"])
        wpeh = pc.tile([128, KT, ROPE], F8)
        nc.sync.dma_start(out=wpeh, in_=d["wpeh"])
        wpel = pc.tile([128, KT, ROPE], F8)
        nc.sync.dma_start(out=wpel, in_=d["wpel"])
        cvth = pc.tile([128, KT, HPC * VD], F8)
        nc.sync.dma_start(out=cvth, in_=d["cvth"])
        cvtl = pc.tile([128, KT, HPC * VD], F8)
        nc.sync.dma_start(out=cvtl, in_=d["cvtl"])
        rotq = pc.tile([128, 128], BF16)
        nc.sync.dma_start(out=rotq, in_=d["rotq"])
        dupx = pc.tile([64, 128], BF16)
        nc.sync.dma_start(out=dupx, in_=d["dupx"])
        duprot = pc.tile([64, 128], BF16)
        nc.sync.dma_start(out=duprot, in_=d["duprot"])
        cosd = pc.tile([128, S], BF16)
        nc.sync.dma_start(out=cosd, in_=d["cos"])
        sind = pc.tile([128, S], BF16)
        nc.sync.dma_start(out=sind, in_=d["sin"])
        eps1 = pc.tile([1, 1], F32)
        nc.vector.memset(eps1, EPS)

        # ---- persistent per-head tensors ----
        qn = [pqkv.tile([128, S], BF16, tag=f"qn{h}", name=f"qn{h}")
              for h in range(HPC)]
        kn = [pqkv.tile([128, S], BF16, tag=f"kn{h}", name=f"kn{h}")
              for h in range(HPC)]
        qpe = pqkv.tile([128, S], BF16, tag="qpe")
        kpd = pqkv.tile([128, S], BF16, tag="kpd")
        vst = pqkv.tile([128, S // 128, HPC * VD], BF16, tag="vst")
        ao = [pqkv.tile([128, S], BF16, tag=f"ao{h}", name=f"ao{h}")
              for h in range(HPC)]
        invqb = pqkv.tile([128, S], BF16, tag="invqb")
        invcb = pqkv.tile([128, S], BF16, tag="invcb")
        invq_row = pqkv.tile([1, NCORES, SEQB], F32R, tag="invq_row")
        invc_row = pqkv.tile([1, NCORES, SEQB], F32R, tag="invc_row")
        invc_col = pqkv.tile([128, NCORES, SEQB // 128], F32, tag="invc_col")

        cc_in = pdram.tile([2, SEQB], F32R)
        cc_out = pdram.tile([NCORES, 2, SEQB], F32R)

        with tc.tile_pool(name="p2sb", bufs=2) as p2sb, \
             tc.tile_pool(name="pp2", bufs=3, space="PSUM") as pp2, \
             tc.tile_pool(name="pp2v", bufs=2, space="PSUM") as pp2v:

            def absorbed(p, wh, wl, ws, xhb, xlb, nt=3):
                """nt-term compensated fp8 DoubleRow accumulation into psum.
                3-term: x8@w_hi + r8@w_hi + x8@w_lo; 2-term drops w_lo."""
                terms = [(xhb, wh), (xlb, wh), (xhb, wl)][:nt]
                for t, (xx, ww) in enumerate(terms):
                    for j in range(NJ):
                        nc.tensor.matmul(
                            p, ww[:, 2 * j:2 * j + 2, ws],
                            xx[:, 2 * j:2 * j + 2, :],
                            start=(t == 0 and j == 0),
                            stop=(t == nt - 1 and j == NJ - 1), perf_mode=DR)

            def block_proj(b):
                cols = bass.ts(b, SB)

                qdst = [qn[0], qn[1]]
                for mt in range(QMT):
                    p_q = pp2.tile([128, SB], F32, tag="mm")
                    absorbed(p_q[:], bth, btl,
                             slice(mt * 128, (mt + 1) * 128), xh[b], xl[b])
                    if mt < 2:
                        nc.any.tensor_copy(qdst[mt][:, cols], p_q[:])
                    else:
                        qpe_u = p2sb.tile([128, SB], BF16, tag="t0")
                        nc.any.tensor_copy(qpe_u[:], p_q[:])
                        p_rq = pp2.tile([128, SB], F32, tag="mm")
                        nc.tensor.matmul(p_rq[:], rotq[:], qpe_u[:],
                                         start=True, stop=True)
                        t1 = p2sb.tile([128, SB], BF16, tag="t1")
                        nc.vector.tensor_mul(t1[:], qpe_u[:], cosd[:, cols])
                        t2 = p2sb.tile([128, SB], BF16, tag="t2")
                        nc.vector.tensor_mul(t2[:], p_rq[:], sind[:, cols])
                        nc.vector.tensor_add(qpe[:, cols], t1[:], t2[:])

                for mt in range(KMT):
                    p_k = pp2.tile([128, SB], F32, tag="mm")
                    absorbed(p_k[:], ckth, cktl,
                             slice(mt * 128, (mt + 1) * 128), xh[b], xl[b],
                             nt=2)
                    nc.any.tensor_copy(kn[mt][:, cols], p_k[:])

                p_pe = pp2.tile([128, SB], F32, tag="mm")
                absorbed(p_pe[:ROPE, :], wpeh, wpel, slice(0, ROPE),
                         xh[b], xl[b], nt=2)
                kpe_u = p2sb.tile([ROPE, SB], BF16, tag="t3")
                nc.any.tensor_copy(kpe_u[:], p_pe[:ROPE, :])
                p_x = pp2.tile([128, SB], F32, tag="mm")
                nc.tensor.matmul(p_x[:], dupx[:], kpe_u[:], start=True, stop=True)
                xb16 = p2sb.tile([128, SB], BF16, tag="t4")
                nc.scalar.activation(xb16[:], p_x[:],
                                     mybir.ActivationFunctionType.Copy)
                p_r = pp2.tile([128, SB], F32, tag="mm")
                nc.tensor.matmul(p_r[:], duprot[:], kpe_u[:], start=True, stop=True)
                rb16 = p2sb.tile([128, SB], BF16, tag="t5")
                nc.scalar.activation(rb16[:], p_r[:],
                                     mybir.ActivationFunctionType.Copy)
                t1 = p2sb.tile([128, SB], BF16, tag="t1")
                nc.vector.tensor_mul(t1[:], xb16[:], cosd[:, cols])
                t2 = p2sb.tile([128, SB], BF16, tag="t2")
                nc.vector.tensor_mul(t2[:], rb16[:], sind[:, cols])
                nc.vector.tensor_add(kpd[:, cols], t1[:], t2[:])

                for t4 in range(SB // 128):
                    sl = slice(t4 * 128, (t4 + 1) * 128)
                    p_v = pp2v.tile([128, HPC * VD], F32, tag="vv")
                    vterms = [(xh[b], cvth), (xl[b], cvth), (xh[b], cvtl)]
                    for t, (xx, ww) in enumerate(vterms):
                        for j in range(NJ):
                            nc.tensor.matmul(
                                p_v[:], xx[:, 2 * j:2 * j + 2, sl],
                                ww[:, 2 * j:2 * j + 2, :],
                                start=(t == 0 and j == 0),
                                stop=(t == 2 and j == NJ - 1), perf_mode=DR)
                    nc.any.tensor_copy(vst[:, b * (SB // 128) + t4, :], p_v[:])

            # block 0 first: its inputs lead the DMA queue
            block_proj(0)

            # ---- phase 1 (stats) in the middle; collective hides ----
            with tc.tile_pool(name="pstats", bufs=1) as pst, \
                 tc.tile_pool(name="p1sb", bufs=3) as p1sb, \
                 tc.tile_pool(name="p1st", bufs=1) as p1st, \
                 tc.tile_pool(name="pp1", bufs=2, space="PSUM") as pp1, \
                 tc.tile_pool(name="pp1s", bufs=1, space="PSUM") as pp1s:
                xs = pst.tile([128, KT, SEQB], F8)
                nc.sync.dma_start(out=xs, in_=d["xs"])
                wqa = pst.tile([128, KT, Q_LORA], F8)
                for ch in range(3):
                    cs = slice(ch * 512, (ch + 1) * 512)
                    nc.sync.dma_start(out=wqa[:, :, cs],
                                      in_=d["wqa"][:, :, cs])
                wkvac = pst.tile([128, KT, KV_LORA], F8)
                nc.sync.dma_start(out=wkvac, in_=d["wkvac"])

                p_qs = pp1s.tile([1, SEQB], F32, tag="stat", name="p_qs")
                for m in range(QLT):
                    p_a = pp1.tile([128, SEQB], F32, tag="acc")
                    for j in range(NJ):
                        nc.tensor.matmul(p_a[:], wqa[:, 2 * j:2 * j + 2,
                                                     m * 128:(m + 1) * 128],
                                         xs[:, 2 * j:2 * j + 2, :],
                                         start=(j == 0), stop=(j == NJ - 1),
                                         perf_mode=DR)
                    ql = p1sb.tile([128, SEQB], BF16, tag="ql")
                    nc.any.tensor_copy(ql[:], p_a[:])
                    sq = p1sb.tile([128, SEQB], BF16, tag="sq")
                    nc.vector.tensor_mul(sq[:], ql[:], ql[:])
                    nc.tensor.matmul(p_qs[:], ones_b[:, 0:1], sq[:],
                                     start=(m == 0), stop=(m == QLT - 1))
                qs_s = p1st.tile([1, SEQB], F32, tag="s1")
                nc.scalar.activation(qs_s[:], p_qs[:], Sqrt,
                                     scale=1.0 / (Q_LORA * WS * WS), bias=eps1[:])
                invq_s = p1st.tile([1, SEQB], F32R, tag="s2")
                nc.vector.reciprocal(invq_s[:], qs_s[:])
                nc.gpsimd.dma_start(out=cc_in[0:1, :], in_=invq_s[:])

                p_cs = pp1s.tile([1, SEQB], F32, tag="stat", name="p_cs")
                for m in range(CT):
                    p_a = pp1.tile([128, SEQB], F32, tag="acc")
                    for j in range(NJ):
                        nc.tensor.matmul(p_a[:], wkvac[:, 2 * j:2 * j + 2,
                                                       m * 128:(m + 1) * 128],
                                         xs[:, 2 * j:2 * j + 2, :],
                                         start=(j == 0), stop=(j == NJ - 1),
                                         perf_mode=DR)
                    cl = p1sb.tile([128, SEQB], BF16, tag="ql")
                    nc.any.tensor_copy(cl[:], p_a[:])
                    sq = p1sb.tile([128, SEQB], BF16, tag="sq")
                    nc.vector.tensor_mul(sq[:], cl[:], cl[:])
                    nc.tensor.matmul(p_cs[:], ones_b[:, 0:1], sq[:],
                                     start=(m == 0), stop=(m == CT - 1))
                cs_s = p1st.tile([1, SEQB], F32, tag="s3")
                nc.scalar.activation(cs_s[:], p_cs[:], Sqrt,
                                     scale=1.0 / (KV_LORA * WS * WS), bias=eps1[:])
                invc_s = p1st.tile([1, SEQB], F32R, tag="s4")
                nc.vector.reciprocal(invc_s[:], cs_s[:])
                nc.gpsimd.dma_start(out=cc_in[1:2, :], in_=invc_s[:])

                nc.gpsimd.collective_compute(
                    "AllGather", mybir.AluOpType.bypass,
                    replica_groups=[list(range(NCORES))],
                    ins=[cc_in[:].opt()], outs=[cc_out[:].opt()])
                nc.sync.dma_start(out=invq_row, in_=cc_out[:, 0, :])
                nc.sync.dma_start(out=invc_row, in_=cc_out[:, 1, :])
                for g in range(NCORES):
                    nc.gpsimd.dma_start(
                        out=invc_col[:, g, :],
                        in_=cc_out[g, 1, :].rearrange("(t p) -> p t", p=128))

            # prefetch blocks 1-2 x tiles, then late-need constants
            for bb in (1, 2):
                cols = bass.ts(bb, SB)
                xh[bb] = pxb.tile([128, KT, SB], F8, tag="xh", name=f"xh{bb}")
                nc.sync.dma_start(out=xh[bb], in_=d["xh"][:, :, cols])
                xl[bb] = pxb.tile([128, KT, SB], F8, tag="xl", name=f"xl{bb}")
                nc.sync.dma_start(out=xl[bb], in_=d["xl"][:, :, cols])
            ones_r = pc.tile([128, 128], F32R)
            nc.sync.dma_start(out=ones_r, in_=d["onesr"])
            msk = pc.tile([128, 4, SB], BF16)
            nc.sync.dma_start(out=msk, in_=d["msk"])
            wo = pc.tile([128, HPC, HIDDEN], BF16)
            nc.sync.dma_start(out=wo, in_=d["wo"])
            for bb in (3,):
                cols = bass.ts(bb, SB)
                xh[bb] = pxb.tile([128, KT, SB], F8, tag="xh", name=f"xh{bb}")
                nc.sync.dma_start(out=xh[bb], in_=d["xh"][:, :, cols])
                xl[bb] = pxb.tile([128, KT, SB], F8, tag="xl", name=f"xl{bb}")
                nc.sync.dma_start(out=xl[bb], in_=d["xl"][:, :, cols])

            def scale_block(b):
                # inv_rms application for block b (needs the AllGather)
                cols = bass.ts(b, SB)
                p_bq = pp2.tile([128, SB], F32, tag="mm")
                nc.tensor.matmul(p_bq[:], ones_r[0:1, :],
                                 invq_row[0:1, 2 * b:2 * b + 2, :],
                                 start=True, stop=True)
                nc.any.tensor_copy(invqb[:, cols], p_bq[:])
                p_bc = pp2.tile([128, SB], F32, tag="mm")
                nc.tensor.matmul(p_bc[:], ones_r[0:1, :],
                                 invc_row[0:1, 2 * b:2 * b + 2, :],
                                 start=True, stop=True)
                nc.any.tensor_copy(invcb[:, cols], p_bc[:])
                nc.vector.tensor_mul(qn[0][:, cols], qn[0][:, cols], invqb[:, cols])
                nc.vector.tensor_mul(qn[1][:, cols], qn[1][:, cols], invqb[:, cols])
                nc.vector.tensor_mul(qpe[:, cols], qpe[:, cols], invqb[:, cols])
                nc.vector.tensor_mul(kn[0][:, cols], kn[0][:, cols], invcb[:, cols])
                nc.vector.tensor_mul(kn[1][:, cols], kn[1][:, cols], invcb[:, cols])
                for st in range(4 * b, 4 * (b + 1)):
                    nc.vector.tensor_scalar_mul(
                        vst[:, st, :], vst[:, st, :],
                        invc_col[:, st // 2, st % 2:st % 2 + 1])

            block_proj(1)
            block_proj(2)
            scale_block(0)
            scale_block(1)
            block_proj(3)
            scale_block(2)
            scale_block(3)

        # ------- phase 3: attention per (block, head) + fused o_proj -------
        # q and k each carry the 2^5 weight pre-scale; fold 2^-10 into Exp.
        ESCALE = SCALE / (WS * WS)
        with tc.tile_pool(name="pexp", bufs=4) as pexp, \
             tc.tile_pool(name="pes", bufs=2) as pes, \
             tc.tile_pool(name="pbn", bufs=2) as pbn, \
             tc.tile_pool(name="pout", bufs=4) as pout, \
             tc.tile_pool(name="ppS", bufs=3, space="PSUM") as ppS, \
             tc.tile_pool(name="ppO", bufs=2, space="PSUM") as ppO, \
             tc.tile_pool(name="ppB", bufs=1, space="PSUM") as ppB, \
             tc.tile_pool(name="ppC", bufs=2, space="PSUM") as ppC:
            for qb in range(NSB):
                qcols = bass.ts(qb, SB)
                nk = 4 * (qb + 1)
                for h in range(HPC):
                    hp = slice(64 * h, 64 * h + 64)
                    p_o = ppO.tile([128, SB], F32, tag="o")
                    esum = [pes.tile([128, SB], F32R, tag=f"es{par}",
                                     name=f"es{par}") for par in range(2)]
                    for ik in range(nk):
                        kc = slice(ik * 128, (ik + 1) * 128)
                        r = ik - 4 * qb
                        # diagonal tiles: only queries >= 128r can attend this
                        # k-tile; trim the matmul/exp width (qb>0 so the
                        # first two chain tiles stay full width)
                        lo = 128 * r if (r >= 1 and qb > 0) else 0
                        qsub = slice(qb * SB + lo, (qb + 1) * SB)
                        sub = slice(lo, SB)
                        p_s = ppS.tile([128, SB], F32, tag="s")
                        nc.tensor.matmul(p_s[:, sub], kn[h][:, kc],
                                         qn[h][:, qsub], start=True, stop=False)
                        nc.tensor.matmul(p_s[:, sub], kpd[hp, kc],
                                         qpe[hp, qsub], start=False, stop=True)
                        e = pexp.tile([128, SB], BF16, tag="e")
                        nc.scalar.activation(e[:, sub], p_s[:, sub], Exp,
                                             scale=ESCALE)
                        if r >= 0:
                            nc.vector.tensor_mul(e[:, sub], e[:, sub],
                                                 msk[:, r, sub])
                        nc.tensor.matmul(p_o[:, sub],
                                         vst[:, ik, h * VD:(h + 1) * VD],
                                         e[:, sub], start=(ik == 0),
                                         stop=(ik == nk - 1))
                        es = esum[ik % 2]
                        if ik < 2:
                            nc.vector.tensor_copy(es[:], e[:])
                        else:
                            nc.vector.tensor_add(es[:, sub], es[:, sub],
                                                 e[:, sub])
                    p_bc = ppB.tile([128, SB], F32, tag="bc")
                    p_d = p_bc[0:1, :]
                    nc.tensor.matmul(p_d, ones_r[:, 0:1], esum[0][:],
                                     start=True, stop=False)
                    nc.tensor.matmul(p_d, ones_r[:, 0:1], esum[1][:],
                                     start=False, stop=True)
                    rec_s = pbn.tile([1, SB], F32R, tag="rec")
                    nc.vector.reciprocal(rec_s[:], p_d)
                    nc.tensor.matmul(p_bc[:], ones_r[0:1, :], rec_s[:],
                                     start=True, stop=True)
                    recb = pbn.tile([128, SB], BF16, tag="recb")
                    nc.any.tensor_copy(recb[:], p_bc[:])
                    nc.vector.tensor_mul(ao[h][:, qcols], p_o[:], recb[:])
                for st in range(qb * (SB // 128), (qb + 1) * (SB // 128)):
                    sc = slice(st * 128, (st + 1) * 128)
                    ot = pout.tile([128, HIDDEN], BF16, tag="ot")
                    fine = (qb == NSB - 1 and st == (qb + 1) * (SB // 128) - 1)
                    for nb in range(HIDDEN // SB):
                        ncols = bass.ts(nb, SB)
                        p_c = ppC.tile([128, SB], F32, tag="c")
                        for h in range(HPC):
                            nc.tensor.matmul(p_c[:], ao[h][:, sc], wo[:, h, ncols],
                                             start=(h == 0), stop=(h == HPC - 1))
                        nc.any.tensor_copy(ot[:, ncols], p_c[:])
                        if fine:
                            # pipeline the final drain with per-chunk DMAs
                            nc.sync.dma_start(out=d["out"][sc, ncols],
                                              in_=ot[:, ncols])
                    if not fine:
                        nc.sync.dma_start(out=d["out"][sc, :], in_=ot[:])


def _host_constants():
    inv_freq = 1.0 / (ROPE_THETA ** (np.arange(0, ROPE, dtype=np.float32)[0::2] / ROPE))
    t = np.arange(S, dtype=np.float32)
    freqs = np.outer(t, inv_freq)
    emb = np.concatenate([freqs, freqs], -1)          # [S, 64]
    cos, sin = np.cos(emb), np.sin(emb)
    cosd = np.concatenate([cos.T, cos.T], 0).astype(np.float32)   # [128, S]
    sind = np.concatenate([sin.T, sin.T], 0).astype(np.float32)

    msk = np.zeros((128, 4, SB), np.float32)
    for r in range(4):
        for p in range(128):
            k_idx = p + 128 * r
            if k_idx < SB:
                msk[p, r, k_idx:] = 1.0               # keep where k <= q

    Q = np.zeros((64, 64), np.float32)
    for i in range(32):
        Q[i, i + 32] = -1.0
        Q[i + 32, i] = 1.0
    P = np.zeros((128, 128), np.float32)
    P[:64, :64] = Q
    P[64:, 64:] = Q
    rotq = P.T.copy()
    D = np.concatenate([np.eye(64, dtype=np.float32)] * 2, 0)     # [128, 64]
    dupx = D.T.copy()                                  # [64, 128]
    duprot = np.concatenate([Q, Q], 0).T.copy()        # [64, 128]
    return cosd, sind, msk, rotq, dupx, duprot


def kernel(hidden_states, w_q_a, q_a_weight, w_q_b, w_kv_a, kv_a_weight,
           w_kv_b, w_o):
    global LAST_RESULT
    import ml_dtypes
    bf16 = ml_dtypes.bfloat16
    f8 = ml_dtypes.float8_e4m3
    if "nc" not in _CACHE:
        _CACHE["nc"] = _build_program()
    nc = _CACHE["nc"]

    def b(a):
        return np.ascontiguousarray(np.asarray(a, np.float32).astype(bf16))

    def e8(a):
        return np.ascontiguousarray(np.asarray(a, np.float32).astype(f8))

    def pre(a):
        """[HIDDEN, F] -> [128, KT, F] partition-major prearrangement."""
        a = np.asarray(a)
        t = a.shape[0] // 128
        return np.ascontiguousarray(
            a.reshape(t, 128, a.shape[1]).transpose(1, 0, 2))

    def split8(a):
        """hi/lo fp8 split of an already-scaled array, prearranged."""
        a = np.asarray(a, np.float32)
        hi = a.astype(f8)
        lo = (a - hi.astype(np.float32)).astype(f8)
        return pre(hi), pre(lo)

    x = np.asarray(hidden_states, np.float32)[0]       # [S, 2048]
    xt = x.T                                           # [2048, S]
    wqa = np.asarray(w_q_a, np.float32)                # [1536, 2048]
    wkva = np.asarray(w_kv_a, np.float32)              # [576, 2048]
    wqb_eff = np.asarray(w_q_b, np.float32) * np.asarray(q_a_weight, np.float32)[None, :]
    wkvb_eff = np.asarray(w_kv_b, np.float32) * np.asarray(kv_a_weight, np.float32)[None, :]
    won = np.asarray(w_o, np.float32)

    cosd, sind, msk, rotq, dupx, duprot = _host_constants()
    onesm = np.ones((128, 128), np.float32)
    xh8, xl8 = split8(xt)
    wpeh8, wpel8 = split8(WS * wkva[KV_LORA:].T)
    shared = {"onesb": b(onesm), "onesr": onesm,
              "xh": xh8, "xl": xl8,
              "wqa_t": pre(e8(WS * wqa.T)), "wkvac_t": pre(e8(WS * wkva[:KV_LORA].T)),
              "wpeh": wpeh8, "wpel": wpel8,
              "cosd": b(cosd), "sind": b(sind), "mask": b(msk),
              "rotq": b(rotq), "dupx": b(dupx), "duprot": b(duprot)}

    in_maps = []
    for c in range(NCORES):
        h0, h1 = HPC * c, HPC * c + 1
        rows_q = np.concatenate(
            [wqb_eff[h0 * QD:h0 * QD + NOPE],
             wqb_eff[h1 * QD:h1 * QD + NOPE],
             wqb_eff[h0 * QD + NOPE:(h0 + 1) * QD],
             wqb_eff[h1 * QD + NOPE:(h1 + 1) * QD]], 0)          # [384, 1536]
        B = rows_q @ wqa                                          # [384, 2048]
        rows_k = np.concatenate(
            [wkvb_eff[h * (NOPE + VD):h * (NOPE + VD) + NOPE] for h in (h0, h1)], 0)
        Ck = rows_k @ wkva[:KV_LORA]
        rows_v = np.concatenate(
            [wkvb_eff[h * (NOPE + VD) + NOPE:(h + 1) * (NOPE + VD)]
             for h in (h0, h1)], 0)
        Cv = rows_v @ wkva[:KV_LORA]
        # v path carries the 2^5 Cv pre-scale: fold 2^-5 into w_o
        wo_t = np.concatenate(
            [won[:, h * VD:(h + 1) * VD] for h in (h0, h1)], 1).T / WS
        bth8, btl8 = split8(WS * B.T)
        ckth8, cktl8 = split8(WS * Ck.T)
        cvth8, cvtl8 = split8(WS * Cv.T)
        im = dict(shared)
        im.update({"bth": bth8, "btl": btl8, "ckth": ckth8, "cktl": cktl8,
                   "cvth": cvth8, "cvtl": cvtl8,
                   "wo_t": pre(b(wo_t)).reshape(128, HPC, HIDDEN),
                   "xs": pre(e8(xt[:, SEQB * c:SEQB * (c + 1)]))})
        in_maps.append(im)

    res = run_bass_kernel_spmd(nc, in_maps, list(range(NCORES)))
    LAST_RESULT = res
    out = np.zeros((S, HIDDEN), np.float32)
    for c in range(NCORES):
        out += res.results[c]["out"].astype(np.float32)
    return out.reshape(1, S, HIDDEN)
